# revision 9
# baseline (speedup 1.0000x reference)
"""GCN (4-layer GCNConv + BatchNorm + ReLU, MLP head) on 8 Trainium2 NeuronCores.

Strategy (see DESIGN.md):
- dst-shard nodes across 8 cores (12500 each); weights replicated.
- Per layer: table = H @ W for all nodes, assembled via AllGather of per-core
  shards; per-edge messages fetched with gpsimd.dma_gather (1024-idx calls,
  4x25000-row table chunks for int16 locals); scaled by edge norm on DVE
  (fp32->bf16); segment-sum by destination via one-hot scatter-matmuls into
  PSUM (S built on DVE by iota-compare); self-loops via per-cell diagonal
  matmuls. BatchNorm stats via free-axis reduction + 8-core AllReduce.
- Z/H kept feature-major ([64, nodes]) in SBUF so BN is per-partition.
"""

import os
import sys
import types

sys.path.insert(0, "/opt/trn_rl_repo")
if "/root/.axon_site" not in sys.path:
    sys.path.insert(0, "/root/.axon_site")

import numpy as np
import ml_dtypes

bf16 = ml_dtypes.bfloat16

def _split_big_waits(nc, limit=1):
    """walrus accepts only one sync-wait per instruction; move overflow waits
    onto preceding EventSemaphore ops on the same engine."""
    import concourse.mybir as mybir
    n_split = 0
    for blk in nc.main_func.blocks:
        i = 0
        while i < len(blk.instructions):
            inst = blk.instructions[i]
            si = inst.sync_info
            if si is not None and si.on_wait and len(si.on_wait) > limit:
                waits = list(si.on_wait)
                overflow, keep = waits[:-limit], waits[-limit:]
                idx = i
                for j in range(0, len(overflow), limit):
                    chunk = overflow[j:j + limit]
                    nop = mybir.InstEventSemaphore(
                        name=nc.get_next_instruction_name(), ins=[], outs=[])
                    nop.engine = inst.engine
                    nop.sync_info = mybir.SyncInfo(on_wait=chunk, on_update=[])
                    nc.register_instruction(nop)
                    blk.instructions.insert(idx, nop)
                    idx += 1
                    i += 1
                si.on_wait = keep
                n_split += 1
            i += 1
    return n_split


def _enable_axon_trace():
    """Register the antenv.axon_hooks NTFF shim (profiling under axon)."""
    import antenv
    if "antenv.axon_hooks" in sys.modules:
        return
    mod = types.ModuleType("antenv.axon_hooks")
    _hook = {"h": None}
    mod.set_axon_ntff_profile_hook = lambda h: _hook.__setitem__("h", h)
    mod.get_axon_ntff_profile_hook = lambda: _hook["h"]
    sys.modules["antenv.axon_hooks"] = mod
    antenv.axon_hooks = mod
    from trn_agent_boot.trn_boot import _ntff_profile_via_ctypes
    h = _ntff_profile_via_ctypes("/opt/axon/libaxon_pjrt.so")
    assert h is not None
    mod.set_axon_ntff_profile_hook(h)
    import concourse.bass_utils as bass_utils
    bass_utils.upload_artifacts = lambda tmpdir: str(tmpdir)


N_NODES = 100000
N_EDGES = 1600000
IN_DIM = 128
HIDDEN = 64
EPS = 1e-5
NCORES = 8
P = 128
NCHUNK = 4
SLAB = 1024          # idx per dma_gather call (HW ring limit ~1024)
SLAB_T = SLAB // P   # 8 tiles per slab


def _wrap_idx16(idx):
    """[n] -> [128, n//16] int16 wrapped in 16 partitions, replicated x8."""
    n16 = len(idx) // 16
    w = np.asarray(idx, np.int16).reshape(n16, 16).T
    return np.tile(w, (8, 1))


def host_prep(x, edge_index, W0, b0, Ws, bs, gammas, betas, hW1, hb1, hW2, hb2,
              n_nodes=N_NODES, ncores=NCORES):
    """Shard + preprocess. Returns (in_maps, meta) for the SPMD program."""
    n_shard = n_nodes // ncores
    cells = (n_shard + P - 1) // P            # dst cells per core
    chunk_rows = n_nodes // NCHUNK            # table chunk height

    src = np.asarray(edge_index[0], np.int64)
    dst = np.asarray(edge_index[1], np.int64)
    deg = np.bincount(dst, minlength=n_nodes).astype(np.float64) + 1.0  # + self loop
    dinv = (1.0 / np.sqrt(deg)).astype(np.float64)
    dinv2 = (dinv * dinv).astype(np.float32)

    core_of = dst // n_shard
    chunk_of = src // chunk_rows

    # per-(core, chunk, cell) edge counts -> shared tile schedule
    dstl_all = dst - core_of * n_shard
    cell_of = dstl_all // P
    counts = np.zeros((ncores, NCHUNK, cells), np.int64)
    np.add.at(counts, (core_of, chunk_of, cell_of), 1)
    tiles_kq = np.maximum(1, -(-counts.max(axis=0) // P))  # [NCHUNK, cells] >= 1

    # slot layout: chunk-major regions; per (chunk, cell) run of tiles_kq*P slots
    region_slots = (tiles_kq.sum(axis=1) * P)
    region_slots_pad = -(-region_slots // SLAB) * SLAB      # pad regions to SLAB
    S_total = int(region_slots_pad.sum())
    T_total = S_total // P

    run_base = np.zeros((NCHUNK, cells), np.int64)
    base = 0
    region_base = []
    for k in range(NCHUNK):
        region_base.append(base)
        for q in range(cells):
            run_base[k, q] = base
            base += int(tiles_kq[k, q]) * P
        base = region_base[k] + int(region_slots_pad[k])
    assert base == S_total

    # tile -> (chunk, cell) schedule + chain start/stop flags (shared all cores)
    tile_owner = [None] * T_total
    for k in range(NCHUNK):
        for q in range(cells):
            t0 = run_base[k, q] // P
            for j in range(int(tiles_kq[k, q])):
                tile_owner[t0 + j] = (k, q, j == 0, j == int(tiles_kq[k, q]) - 1)

    in_maps = []
    for c in range(ncores):
        m = core_of == c
        src_c = src[m]
        dstl_c = dstl_all[m]
        k_c = chunk_of[m]
        q_c = cell_of[m]
        norm_c = (dinv[src_c] * dinv[dst[m]]).astype(np.float32)

        order = np.lexsort((dstl_c, q_c, k_c))
        src_c, dstl_c, k_c, q_c, norm_c = (a[order] for a in (src_c, dstl_c, k_c, q_c, norm_c))

        idx_slots = np.zeros(S_total, np.int16)
        dl_slots = np.full(S_total, 255.0, np.float32)
        nm_slots = np.zeros(S_total, np.float32)
        # place each (k, q) run
        ptr = 0
        for k in range(NCHUNK):
            for q in range(cells):
                n_kq = int(counts[c, k, q])
                sl = slice(ptr, ptr + n_kq)
                b0_ = int(run_base[k, q])
                idx_slots[b0_:b0_ + n_kq] = (src_c[sl] - k * chunk_rows).astype(np.int16)
                dl_slots[b0_:b0_ + n_kq] = (dstl_c[sl] - q * P).astype(np.float32)
                nm_slots[b0_:b0_ + n_kq] = norm_c[sl]
                ptr += n_kq
        assert ptr == len(src_c)

        # slot s -> [s % 128, s // 128]
        idxw = np.concatenate(
            [_wrap_idx16(idx_slots[region_base[k]:region_base[k] + int(region_slots_pad[k])])
             for k in range(NCHUNK)], axis=1)
        # NOTE: wrapped layout is per-16 within each slab of its region; dma_gather
        # unwraps [16, :] as (s p): slot i of a call -> idx at [i % 16, i // 16] of the
        # call's column slice. Building per-region keeps call slices aligned.
        dl_arr = dl_slots.reshape(T_total, P).T.astype(bf16)      # [128, T]
        nm_arr = nm_slots.reshape(T_total, P).T.astype(np.float32)  # [128, T] fp32

        # D2: [128, cells*128] bf16; [r, q*128 + j] = (r == j) * dinv2[shard node q*128+r]
        d2 = np.zeros((P, cells * P), np.float32)
        for q in range(cells):
            nmax = min(P, n_shard - q * P)
            rr = np.arange(nmax)
            d2[rr, q * P + rr] = dinv2[c * n_shard + q * P + rr]
        # x shard transposed, padded to cells*128 columns
        xT = np.zeros((IN_DIM, cells * P), np.float32)
        xT[:, :n_shard] = np.asarray(x, np.float32)[c * n_shard:(c + 1) * n_shard].T

        in_maps.append({
            "xT": xT,
            "idxw": np.ascontiguousarray(idxw),
            "dstl": np.ascontiguousarray(dl_arr),
            "nrm": np.ascontiguousarray(nm_arr),
            "d2": d2.astype(bf16),
            "w0": np.asarray(W0, np.float32),
            "wl": np.asarray(Ws, np.float32).astype(bf16),
            "gb": np.concatenate([np.asarray(gammas, np.float32).T,
                                  np.asarray(betas, np.float32).T], axis=1),  # [64, 8]
            "hw1": np.asarray(hW1, np.float32).astype(bf16),
            "hb1": np.asarray(hb1, np.float32).reshape(HIDDEN, 1),
            "hw2": np.asarray(hW2, np.float32).astype(bf16).reshape(HIDDEN, 1),
            "iota": np.tile(np.arange(P, dtype=np.float32).reshape(1, P), (P, 1)).astype(bf16),
        })

    meta = dict(n_nodes=n_nodes, n_shard=n_shard, cells=cells, chunk_rows=chunk_rows,
                S_total=S_total, T_total=T_total, tile_owner=tile_owner,
                region_base=region_base,
                region_slots_pad=[int(v) for v in region_slots_pad],
                hb2=float(np.asarray(hb2).reshape(-1)[0]))
    return in_maps, meta


def build_program(meta, ncores=NCORES):
    import concourse.bass as bass
    import concourse.bacc as bacc
    import concourse.mybir as mybir
    import concourse.tile as tile

    n_nodes = meta["n_nodes"]
    n_shard = meta["n_shard"]
    cells = meta["cells"]
    chunk_rows = meta["chunk_rows"]
    S_total = meta["S_total"]
    T_total = meta["T_total"]
    tile_owner = meta["tile_owner"]
    region_base = meta["region_base"]
    region_slots_pad = meta["region_slots_pad"]
    hb2 = meta["hb2"]
    NPC = cells * P  # padded shard width

    f32 = mybir.dt.float32
    b16 = mybir.dt.bfloat16
    add_ = mybir.AluOpType.add
    mult_ = mybir.AluOpType.mult
    iseq_ = mybir.AluOpType.is_equal
    max_ = mybir.AluOpType.max
    subtract_ = mybir.AluOpType.subtract

    nc = bacc.Bacc(num_devices=ncores, num_swdge_queues=2)
    xT_in = nc.declare_dram_parameter("xT", [IN_DIM, NPC], f32, isOutput=False)
    idx_in = nc.declare_dram_parameter("idxw", [P, S_total // 16], mybir.dt.int16, isOutput=False)
    dstl_in = nc.declare_dram_parameter("dstl", [P, T_total], b16, isOutput=False)
    nrm_in = nc.declare_dram_parameter("nrm", [P, T_total], f32, isOutput=False)
    d2_in = nc.declare_dram_parameter("d2", [P, cells * P], b16, isOutput=False)
    w0_in = nc.declare_dram_parameter("w0", [IN_DIM, HIDDEN], f32, isOutput=False)
    wl_in = nc.declare_dram_parameter("wl", [3, HIDDEN, HIDDEN], b16, isOutput=False)
    gb_in = nc.declare_dram_parameter("gb", [HIDDEN, 8], f32, isOutput=False)
    hw1_in = nc.declare_dram_parameter("hw1", [HIDDEN, HIDDEN], b16, isOutput=False)
    hb1_in = nc.declare_dram_parameter("hb1", [HIDDEN, 1], f32, isOutput=False)
    hw2_in = nc.declare_dram_parameter("hw2", [HIDDEN, 1], b16, isOutput=False)
    iota_in = nc.declare_dram_parameter("iota", [P, P], b16, isOutput=False)
    y_out = nc.declare_dram_parameter("y", [n_shard, 1], f32, isOutput=True)

    with tile.TileContext(nc, num_cores=ncores) as tc:
        with (
            tc.tile_pool(name="dram", bufs=1, space="DRAM") as dpool,
            tc.tile_pool(name="const", bufs=1) as cpool,
            tc.tile_pool(name="state", bufs=1) as spool,
            tc.tile_pool(name="mslab", bufs=6) as mpool,
            tc.tile_pool(name="sslab", bufs=6) as sbpool,
            tc.tile_pool(name="ttile", bufs=4) as tpool,
            tc.tile_pool(name="psum", bufs=4, space="PSUM") as ppool,
            tc.tile_pool(name="psum2", bufs=2, space="PSUM") as ppool2,
        ):
            ag_in = dpool.tile([n_shard, HIDDEN], f32)
            table = dpool.tile([n_nodes, HIDDEN], f32)
            bn_in = dpool.tile([HIDDEN, 2], f32)
            bn_out = dpool.tile([HIDDEN, 2], f32)

            # ---- constants to SBUF ----
            idxs = cpool.tile([P, S_total // 16], mybir.dt.int16)
            nc.sync.dma_start(out=idxs[:], in_=idx_in[:])
            dstl = cpool.tile([P, T_total], b16)
            nc.sync.dma_start(out=dstl[:], in_=dstl_in[:])
            nrm = cpool.tile([P, T_total], f32)
            nc.sync.dma_start(out=nrm[:], in_=nrm_in[:])
            d2 = cpool.tile([P, cells, P], b16)
            nc.sync.dma_start(out=d2[:], in_=d2_in.rearrange("p (q j) -> p q j", j=P))
            w0 = cpool.tile([IN_DIM, HIDDEN], f32)
            nc.sync.dma_start(out=w0[:], in_=w0_in[:])
            wls = []
            for i in range(3):
                wli = cpool.tile([HIDDEN, HIDDEN], b16, name=f"wl{i}")
                nc.sync.dma_start(out=wli[:], in_=wl_in[i])
                wls.append(wli)
            gb = cpool.tile([HIDDEN, 8], f32)
            nc.sync.dma_start(out=gb[:], in_=gb_in[:])
            hw1 = cpool.tile([HIDDEN, HIDDEN], b16)
            nc.sync.dma_start(out=hw1[:], in_=hw1_in[:])
            hb1 = cpool.tile([HIDDEN, 1], f32)
            nc.sync.dma_start(out=hb1[:], in_=hb1_in[:])
            hw2 = cpool.tile([HIDDEN, 1], b16)
            nc.sync.dma_start(out=hw2[:], in_=hw2_in[:])
            iota = cpool.tile([P, P], b16)
            nc.sync.dma_start(out=iota[:], in_=iota_in[:])

            # ---- state ----
            ZT = spool.tile([HIDDEN, NPC], f32)
            HT = spool.tile([HIDDEN, NPC], b16)
            shard_b = spool.tile([P, cells, HIDDEN], b16)
            stats = spool.tile([HIDDEN, 8], f32)  # sum, sumsq, mean, ex2, var, rstd, scale, shift
            NSQ = (NPC + 1023) // 1024
            sqp = spool.tile([HIDDEN, NSQ], f32)
            sq = spool.tile([HIDDEN, 1024], b16)

            for layer in range(4):
                # ---------- table phase: shard tiles + AllGather ----------
                for t in range(cells):
                    pt = ppool2.tile([P, HIDDEN], f32, tag="aux")
                    if layer == 0:
                        xt = tpool.tile([IN_DIM, P], f32, tag="xt")
                        nc.sync.dma_start(out=xt[:], in_=xT_in[:, t * P:(t + 1) * P])
                        nc.tensor.matmul(pt[:], lhsT=xt[:], rhs=w0[:], start=True, stop=True)
                    else:
                        nc.tensor.matmul(pt[:], lhsT=HT[:, t * P:(t + 1) * P],
                                         rhs=wls[layer - 1][:], start=True, stop=True)
                    tt = tpool.tile([P, HIDDEN], f32, tag="tt")
                    nc.vector.tensor_copy(out=tt[:], in_=pt[:])
                    nc.vector.tensor_copy(out=shard_b[:, t, :], in_=pt[:])
                    hi = min(P, n_shard - t * P)
                    nc.sync.dma_start(out=ag_in[t * P:t * P + hi, :], in_=tt[:hi, :])
                nc.gpsimd.collective_compute(
                    "AllGather", mybir.AluOpType.bypass,
                    ins=[ag_in.opt()], outs=[table.opt()],
                    replica_groups=[list(range(ncores))],
                )

                # ---------- gather + scatter ----------
                open_psum = {}
                for k in range(NCHUNK):
                    rb_slots = region_base[k]
                    ncalls = region_slots_pad[k] // SLAB
                    for j in range(ncalls):
                        s0 = rb_slots + j * SLAB          # first slot of call
                        t0 = s0 // P                       # first tile index
                        col0 = s0 // 16                    # idx col base
                        mt = mpool.tile([P, SLAB_T, HIDDEN], f32, tag="m")
                        nc.gpsimd.dma_gather(
                            out_ap=mt[:],
                            in_ap=table[k * chunk_rows:(k + 1) * chunk_rows, :],
                            idxs_ap=idxs[:, col0:col0 + SLAB // 16],
                            num_idxs=SLAB,
                            num_idxs_reg=SLAB,
                            elem_size=HIDDEN,
                            queue_num=(k * 1000 + j) % 2,
                        )
                        mb = mpool.tile([P, SLAB_T, HIDDEN], b16, tag="mb")
                        nc.vector.tensor_tensor(
                            out=mb[:], in0=mt[:],
                            in1=nrm[:, t0:t0 + SLAB_T].rearrange("p (t o) -> p t o", o=1).to_broadcast([P, SLAB_T, HIDDEN]),
                            op=mult_,
                        )
                        st = sbpool.tile([P, SLAB_T, P], b16, tag="s")
                        nc.vector.tensor_tensor(
                            out=st[:],
                            in0=dstl[:, t0:t0 + SLAB_T].rearrange("p (t o) -> p t o", o=1).to_broadcast([P, SLAB_T, P]),
                            in1=iota[:].rearrange("p (o f) -> p o f", o=1).to_broadcast([P, SLAB_T, P]),
                            op=iseq_,
                        )
                        for ti in range(SLAB_T):
                            owner = tile_owner[t0 + ti] if t0 + ti < T_total else None
                            if owner is None:
                                continue
                            kk, q, is_first, is_last = owner
                            assert kk == k
                            if is_first:
                                zc = ppool.tile([HIDDEN, P], f32, tag="zc", name=f"zc_{layer}_{k}_{q}")
                                open_psum[q] = zc
                                if k == 0:
                                    nc.tensor.matmul(zc[:], lhsT=shard_b[:, q, :], rhs=d2[:, q, :],
                                                     start=True, stop=False)
                            zc = open_psum[q]
                            nc.tensor.matmul(zc[:], lhsT=mb[:, ti, :], rhs=st[:, ti, :],
                                             start=(is_first and k != 0), stop=is_last)
                            if is_last:
                                if k == 0:
                                    nc.vector.tensor_copy(out=ZT[:, q * P:(q + 1) * P], in_=zc[:])
                                else:
                                    nc.vector.tensor_tensor(out=ZT[:, q * P:(q + 1) * P],
                                                            in0=ZT[:, q * P:(q + 1) * P],
                                                            in1=zc[:], op=add_)
                                del open_psum[q]

                # ---------- BatchNorm + ReLU ----------
                nc.vector.reduce_sum(stats[:, 0:1], ZT[:], axis=mybir.AxisListType.X)
                for sc in range(NSQ):
                    w = min(1024, NPC - sc * 1024)
                    nc.vector.tensor_tensor(out=sq[:, :w], in0=ZT[:, sc * 1024:sc * 1024 + w],
                                            in1=ZT[:, sc * 1024:sc * 1024 + w], op=mult_)
                    nc.vector.reduce_sum(sqp[:, sc:sc + 1], sq[:, :w], axis=mybir.AxisListType.X)
                nc.vector.reduce_sum(stats[:, 1:2], sqp[:], axis=mybir.AxisListType.X)
                bt = tpool.tile([HIDDEN, 2], f32, tag="bt")
                nc.vector.tensor_copy(out=bt[:], in_=stats[:, 0:2])
                nc.sync.dma_start(out=bn_in[:], in_=bt[:])
                nc.gpsimd.collective_compute(
                    "AllReduce", add_,
                    ins=[bn_in.opt()], outs=[bn_out.opt()],
                    replica_groups=[list(range(ncores))],
                )
                nc.sync.dma_start(out=stats[:, 0:2], in_=bn_out[:])
                inv_n = 1.0 / float(n_nodes)
                nc.vector.tensor_scalar_mul(stats[:, 2:3], stats[:, 0:1], inv_n)   # mean
                nc.vector.tensor_scalar_mul(stats[:, 3:4], stats[:, 1:2], inv_n)   # E[x^2]
                nc.vector.tensor_tensor(out=stats[:, 4:5], in0=stats[:, 2:3],
                                        in1=stats[:, 2:3], op=mult_)               # mean^2
                nc.vector.tensor_tensor(out=stats[:, 4:5], in0=stats[:, 3:4],
                                        in1=stats[:, 4:5], op=subtract_)           # var
                nc.vector.tensor_scalar_add(stats[:, 4:5], stats[:, 4:5], EPS)
                nc.scalar.activation(stats[:, 5:6], stats[:, 4:5],
                                     mybir.ActivationFunctionType.Sqrt)             # std
                nc.vector.reciprocal(stats[:, 5:6], stats[:, 5:6])                  # rstd
                nc.vector.tensor_tensor(out=stats[:, 6:7], in0=gb[:, layer:layer + 1],
                                        in1=stats[:, 5:6], op=mult_)               # scale
                nc.vector.tensor_tensor(out=stats[:, 7:8], in0=stats[:, 2:3],
                                        in1=stats[:, 6:7], op=mult_)               # mean*scale
                nc.vector.tensor_tensor(out=stats[:, 7:8], in0=gb[:, 4 + layer:5 + layer],
                                        in1=stats[:, 7:8], op=subtract_)           # shift
                nc.vector.scalar_tensor_tensor(
                    out=HT[:], in0=ZT[:], scalar=stats[:, 6:7],
                    in1=stats[:, 7:8].to_broadcast([HIDDEN, NPC]),
                    op0=mult_, op1=add_,
                )
                nc.vector.tensor_scalar(HT[:], HT[:], 0.0, None, op0=max_)

            # ---------- head ----------
            for t in range(cells):
                g = ppool2.tile([HIDDEN, P], f32, tag="aux")
                nc.tensor.matmul(g[:], lhsT=hw1[:], rhs=HT[:, t * P:(t + 1) * P],
                                 start=True, stop=True)
                gs = tpool.tile([HIDDEN, P], b16, tag="gs")
                nc.vector.tensor_tensor(out=gs[:], in0=g[:],
                                        in1=hb1[:].to_broadcast([HIDDEN, P]), op=add_)
                nc.vector.tensor_scalar(gs[:], gs[:], 0.0, None, op0=max_)
                o = ppool2.tile([1, P], f32, tag="aux")
                nc.tensor.matmul(o[:], lhsT=hw2[:], rhs=gs[:], start=True, stop=True)
                ot = tpool.tile([1, P], f32, tag="ot")
                nc.vector.tensor_scalar(ot[:], o[:], hb2, None, op0=add_)
                hi = min(P, n_shard - t * P)
                nc.sync.dma_start(out=y_out[t * P:t * P + hi, :].rearrange("n o -> o n"),
                                  in_=ot[:, :hi])

    nc.finalize()
    _split_big_waits(nc)
    return nc


_CACHE = {}


def kernel(x, edge_index, W0, b0, Ws, bs, gammas, betas, hW1, hb1, hW2, hb2):
    sys.path.insert(0, os.path.dirname(os.path.abspath(__file__)))
    from concourse.bass_utils import run_bass_kernel_spmd

    in_maps, meta = host_prep(x, edge_index, W0, b0, Ws, bs, gammas, betas,
                              hW1, hb1, hW2, hb2)
    key = meta["S_total"]
    if key not in _CACHE:
        _CACHE[key] = build_program(meta)
    nc = _CACHE[key]

    trace = os.environ.get("BASS_GCN_TRACE", "0") == "1"
    kwargs = {}
    if trace:
        import tempfile
        _enable_axon_trace()
        kwargs = dict(trace=True, tmpdir=tempfile.mkdtemp())
    res = run_bass_kernel_spmd(nc, in_maps, core_ids=list(range(NCORES)), **kwargs)
    if trace:
        kernel.last_exec_time_ns = res.exec_time_ns
        kernel.last_trace_dir = kwargs["tmpdir"]
    out = np.concatenate([res.results[c]["y"] for c in range(NCORES)], axis=0)
    return out.astype(np.float32)


# revision 11
# speedup vs baseline: 1.3174x; 1.3174x over previous
"""GCN (4-layer GCNConv + BatchNorm + ReLU, MLP head) on 8 Trainium2 NeuronCores.

Strategy (see DESIGN.md):
- dst-shard nodes across 8 cores (12500 each); weights replicated.
- Per layer: table = H @ W for all nodes, assembled via AllGather of per-core
  shards; per-edge messages fetched with gpsimd.dma_gather (1024-idx calls,
  4x25000-row table chunks for int16 locals); scaled by edge norm on DVE
  (fp32->bf16); segment-sum by destination via one-hot scatter-matmuls into
  PSUM (S built on DVE by iota-compare); self-loops via per-cell diagonal
  matmuls. BatchNorm stats via free-axis reduction + 8-core AllReduce.
- Z/H kept feature-major ([64, nodes]) in SBUF so BN is per-partition.
"""

import os
import sys
import types

sys.path.insert(0, "/opt/trn_rl_repo")
if "/root/.axon_site" not in sys.path:
    sys.path.insert(0, "/root/.axon_site")

import numpy as np
import ml_dtypes

bf16 = ml_dtypes.bfloat16

def _split_big_waits(nc, limit=1):
    """walrus accepts only one sync-wait per instruction; move overflow waits
    onto preceding EventSemaphore ops on the same engine."""
    import concourse.mybir as mybir
    n_split = 0
    for blk in nc.main_func.blocks:
        i = 0
        while i < len(blk.instructions):
            inst = blk.instructions[i]
            si = inst.sync_info
            if si is not None and si.on_wait and len(si.on_wait) > limit:
                waits = list(si.on_wait)
                overflow, keep = waits[:-limit], waits[-limit:]
                idx = i
                for j in range(0, len(overflow), limit):
                    chunk = overflow[j:j + limit]
                    nop = mybir.InstEventSemaphore(
                        name=nc.get_next_instruction_name(), ins=[], outs=[])
                    nop.engine = inst.engine
                    nop.sync_info = mybir.SyncInfo(on_wait=chunk, on_update=[])
                    nc.register_instruction(nop)
                    blk.instructions.insert(idx, nop)
                    idx += 1
                    i += 1
                si.on_wait = keep
                n_split += 1
            i += 1
    return n_split


def _enable_axon_trace():
    """Register the antenv.axon_hooks NTFF shim (profiling under axon)."""
    import antenv
    if "antenv.axon_hooks" in sys.modules:
        return
    mod = types.ModuleType("antenv.axon_hooks")
    _hook = {"h": None}
    mod.set_axon_ntff_profile_hook = lambda h: _hook.__setitem__("h", h)
    mod.get_axon_ntff_profile_hook = lambda: _hook["h"]
    sys.modules["antenv.axon_hooks"] = mod
    antenv.axon_hooks = mod
    from trn_agent_boot.trn_boot import _ntff_profile_via_ctypes
    h = _ntff_profile_via_ctypes("/opt/axon/libaxon_pjrt.so")
    assert h is not None
    mod.set_axon_ntff_profile_hook(h)
    import concourse.bass_utils as bass_utils
    bass_utils.upload_artifacts = lambda tmpdir: str(tmpdir)


N_NODES = 100000
N_EDGES = 1600000
IN_DIM = 128
HIDDEN = 64
EPS = 1e-5
NCORES = 8
P = 128
NCHUNK = 4
SLAB = 1024          # idx per dma_gather call (HW ring limit ~1024)
SLAB_T = SLAB // P   # 8 tiles per slab


def _wrap_idx16(idx):
    """[n] -> [128, n//16] int16 wrapped in 16 partitions, replicated x8."""
    n16 = len(idx) // 16
    w = np.asarray(idx, np.int16).reshape(n16, 16).T
    return np.tile(w, (8, 1))


def host_prep(x, edge_index, W0, b0, Ws, bs, gammas, betas, hW1, hb1, hW2, hb2,
              n_nodes=N_NODES, ncores=NCORES):
    """Shard + preprocess. Returns (in_maps, meta) for the SPMD program."""
    n_shard = n_nodes // ncores
    cells = (n_shard + P - 1) // P            # dst cells per core
    chunk_rows = n_nodes // NCHUNK            # table chunk height

    src = np.asarray(edge_index[0], np.int64)
    dst = np.asarray(edge_index[1], np.int64)
    deg = np.bincount(dst, minlength=n_nodes).astype(np.float64) + 1.0  # + self loop
    dinv = (1.0 / np.sqrt(deg)).astype(np.float64)
    dinv2 = (dinv * dinv).astype(np.float32)

    core_of = dst // n_shard
    chunk_of = src // chunk_rows

    BLK = 64                                   # dsts per budget block
    nblk = (n_shard + BLK - 1) // BLK          # blocks per core (per chunk)
    dstl_all = dst - core_of * n_shard
    blk_of = dstl_all // BLK
    counts = np.zeros((ncores, NCHUNK, nblk), np.int64)
    np.add.at(counts, (core_of, chunk_of, blk_of), 1)
    budget = np.maximum(counts.max(axis=0), P)  # [NCHUNK, nblk] slots, >= 128

    # slot layout: chunk-major regions; block b gets budget[k, b] slots
    region_slots = budget.sum(axis=1)
    region_slots_pad = -(-region_slots // SLAB) * SLAB
    S_total = int(region_slots_pad.sum())
    T_total = S_total // P

    blk_base = np.zeros((NCHUNK, nblk), np.int64)
    base = 0
    region_base = []
    for k in range(NCHUNK):
        region_base.append(base)
        for b in range(nblk):
            blk_base[k, b] = base
            base += int(budget[k, b])
        base = region_base[k] + int(region_slots_pad[k])
    assert base == S_total

    # per-tile dst window (uniform across cores): win0 = BLK * block(first slot),
    # clamped; None for region-tail tiles holding no block slots
    cells = (n_shard + P - 1) // P
    NPC = cells * P
    slot_block = np.full(S_total, -1, np.int64)   # block of each slot (k-local)
    for k in range(NCHUNK):
        for b in range(nblk):
            s0 = int(blk_base[k, b])
            slot_block[s0:s0 + int(budget[k, b])] = b
    tile_win = []
    for t in range(T_total):
        b = slot_block[t * P]
        if b < 0:
            # tail tile; may still contain trailing block slots? (blocks end before
            # region pad) -> no slots
            tile_win.append(None)
        else:
            tile_win.append(int(min(b * BLK, NPC - P)))

    in_maps = []
    for c in range(ncores):
        m = core_of == c
        src_c = src[m]
        dstl_c = dstl_all[m]
        k_c = chunk_of[m]
        b_c = blk_of[m]
        norm_c = (dinv[src_c] * dinv[dst[m]]).astype(np.float32)

        order = np.lexsort((dstl_c, b_c, k_c))
        src_c, dstl_c, k_c, b_c, norm_c = (a[order] for a in (src_c, dstl_c, k_c, b_c, norm_c))

        idx_slots = np.zeros(S_total, np.int16)
        dl_slots = np.full(S_total, 255.0, np.float32)
        nm_slots = np.zeros(S_total, np.float32)
        ptr = 0
        for k in range(NCHUNK):
            cnts = counts[c, k]
            for b in range(nblk):
                n_kb = int(cnts[b])
                sl = slice(ptr, ptr + n_kb)
                s0 = int(blk_base[k, b])
                idx_slots[s0:s0 + n_kb] = (src_c[sl] - k * chunk_rows).astype(np.int16)
                # dst_local relative to the WINDOW of each slot's tile
                ss = np.arange(s0, s0 + n_kb)
                w0 = np.array([tile_win[t] for t in ss // P], np.int64)
                dl_slots[s0:s0 + n_kb] = (dstl_c[sl] - w0).astype(np.float32)
                nm_slots[s0:s0 + n_kb] = norm_c[sl]
                ptr += n_kb
        assert ptr == len(src_c)
        assert (dl_slots >= 0).all() and (dl_slots <= 255).all()

        # slot s -> [s % 128, s // 128]
        idxw = np.concatenate(
            [_wrap_idx16(idx_slots[region_base[k]:region_base[k] + int(region_slots_pad[k])])
             for k in range(NCHUNK)], axis=1)
        # NOTE: wrapped layout is per-16 within each slab of its region; dma_gather
        # unwraps [16, :] as (s p): slot i of a call -> idx at [i % 16, i // 16] of the
        # call's column slice. Building per-region keeps call slices aligned.
        dl_arr = dl_slots.reshape(T_total, P).T.astype(bf16)      # [128, T]
        nm_arr = nm_slots.reshape(T_total, P).T.astype(np.float32)  # [128, T] fp32

        # D2: [128, cells*128] bf16; [r, q*128 + j] = (r == j) * dinv2[shard node q*128+r]
        d2 = np.zeros((P, cells * P), np.float32)
        for q in range(cells):
            nmax = min(P, n_shard - q * P)
            rr = np.arange(nmax)
            d2[rr, q * P + rr] = dinv2[c * n_shard + q * P + rr]
        # x shard transposed, padded to cells*128 columns
        xT = np.zeros((IN_DIM, cells * P), np.float32)
        xT[:, :n_shard] = np.asarray(x, np.float32)[c * n_shard:(c + 1) * n_shard].T

        in_maps.append({
            "xT": xT,
            "idxw": np.ascontiguousarray(idxw),
            "dstl": np.ascontiguousarray(dl_arr),
            "nrm": np.ascontiguousarray(nm_arr),
            "d2": d2.astype(bf16),
            "w0": np.asarray(W0, np.float32),
            "wl": np.asarray(Ws, np.float32).astype(bf16),
            "gb": np.concatenate([np.asarray(gammas, np.float32).T,
                                  np.asarray(betas, np.float32).T], axis=1),  # [64, 8]
            "hw1": np.asarray(hW1, np.float32).astype(bf16),
            "hb1": np.asarray(hb1, np.float32).reshape(HIDDEN, 1),
            "hw2": np.asarray(hW2, np.float32).astype(bf16).reshape(HIDDEN, 1),
            "iota": np.tile(np.arange(P, dtype=np.float32).reshape(1, P), (P, 1)).astype(bf16),
        })

    meta = dict(n_nodes=n_nodes, n_shard=n_shard, cells=cells, chunk_rows=chunk_rows,
                S_total=S_total, T_total=T_total, tile_win=tile_win,
                region_base=region_base,
                region_slots_pad=[int(v) for v in region_slots_pad],
                hb2=float(np.asarray(hb2).reshape(-1)[0]))
    return in_maps, meta


def build_program(meta, ncores=NCORES):
    import concourse.bass as bass
    import concourse.bacc as bacc
    import concourse.mybir as mybir
    import concourse.tile as tile

    n_nodes = meta["n_nodes"]
    n_shard = meta["n_shard"]
    cells = meta["cells"]
    chunk_rows = meta["chunk_rows"]
    S_total = meta["S_total"]
    T_total = meta["T_total"]
    tile_win = meta["tile_win"]
    region_base = meta["region_base"]
    region_slots_pad = meta["region_slots_pad"]
    hb2 = meta["hb2"]
    NPC = cells * P  # padded shard width

    f32 = mybir.dt.float32
    b16 = mybir.dt.bfloat16
    add_ = mybir.AluOpType.add
    mult_ = mybir.AluOpType.mult
    iseq_ = mybir.AluOpType.is_equal
    max_ = mybir.AluOpType.max
    subtract_ = mybir.AluOpType.subtract

    nc = bacc.Bacc(num_devices=ncores, num_swdge_queues=2)
    xT_in = nc.declare_dram_parameter("xT", [IN_DIM, NPC], f32, isOutput=False)
    idx_in = nc.declare_dram_parameter("idxw", [P, S_total // 16], mybir.dt.int16, isOutput=False)
    dstl_in = nc.declare_dram_parameter("dstl", [P, T_total], b16, isOutput=False)
    nrm_in = nc.declare_dram_parameter("nrm", [P, T_total], f32, isOutput=False)
    d2_in = nc.declare_dram_parameter("d2", [P, cells * P], b16, isOutput=False)
    w0_in = nc.declare_dram_parameter("w0", [IN_DIM, HIDDEN], f32, isOutput=False)
    wl_in = nc.declare_dram_parameter("wl", [3, HIDDEN, HIDDEN], b16, isOutput=False)
    gb_in = nc.declare_dram_parameter("gb", [HIDDEN, 8], f32, isOutput=False)
    hw1_in = nc.declare_dram_parameter("hw1", [HIDDEN, HIDDEN], b16, isOutput=False)
    hb1_in = nc.declare_dram_parameter("hb1", [HIDDEN, 1], f32, isOutput=False)
    hw2_in = nc.declare_dram_parameter("hw2", [HIDDEN, 1], b16, isOutput=False)
    iota_in = nc.declare_dram_parameter("iota", [P, P], b16, isOutput=False)
    y_out = nc.declare_dram_parameter("y", [n_shard, 1], f32, isOutput=True)

    with tile.TileContext(nc, num_cores=ncores) as tc:
        with (
            tc.tile_pool(name="dram", bufs=1, space="DRAM") as dpool,
            tc.tile_pool(name="const", bufs=1) as cpool,
            tc.tile_pool(name="state", bufs=1) as spool,
            tc.tile_pool(name="mslab", bufs=6) as mpool,
            tc.tile_pool(name="sslab", bufs=6) as sbpool,
            tc.tile_pool(name="ttile", bufs=4) as tpool,
            tc.tile_pool(name="psum", bufs=4, space="PSUM") as ppool,
            tc.tile_pool(name="psum2", bufs=2, space="PSUM") as ppool2,
        ):
            ag_in = dpool.tile([n_shard, HIDDEN], f32)
            table = dpool.tile([n_nodes, HIDDEN], f32)
            bn_in = dpool.tile([HIDDEN, 2], f32)
            bn_out = dpool.tile([HIDDEN, 2], f32)

            # ---- constants to SBUF ----
            idxs = cpool.tile([P, S_total // 16], mybir.dt.int16)
            nc.sync.dma_start(out=idxs[:], in_=idx_in[:])
            dstl = cpool.tile([P, T_total], b16)
            nc.sync.dma_start(out=dstl[:], in_=dstl_in[:])
            nrm = cpool.tile([P, T_total], f32)
            nc.sync.dma_start(out=nrm[:], in_=nrm_in[:])
            d2 = cpool.tile([P, cells, P], b16)
            nc.sync.dma_start(out=d2[:], in_=d2_in.rearrange("p (q j) -> p q j", j=P))
            w0 = cpool.tile([IN_DIM, HIDDEN], f32)
            nc.sync.dma_start(out=w0[:], in_=w0_in[:])
            wls = []
            for i in range(3):
                wli = cpool.tile([HIDDEN, HIDDEN], b16, name=f"wl{i}")
                nc.sync.dma_start(out=wli[:], in_=wl_in[i])
                wls.append(wli)
            gb = cpool.tile([HIDDEN, 8], f32)
            nc.sync.dma_start(out=gb[:], in_=gb_in[:])
            hw1 = cpool.tile([HIDDEN, HIDDEN], b16)
            nc.sync.dma_start(out=hw1[:], in_=hw1_in[:])
            hb1 = cpool.tile([HIDDEN, 1], f32)
            nc.sync.dma_start(out=hb1[:], in_=hb1_in[:])
            hw2 = cpool.tile([HIDDEN, 1], b16)
            nc.sync.dma_start(out=hw2[:], in_=hw2_in[:])
            iota = cpool.tile([P, P], b16)
            nc.sync.dma_start(out=iota[:], in_=iota_in[:])

            # ---- state ----
            ZT = spool.tile([HIDDEN, NPC], f32)
            HT = spool.tile([HIDDEN, NPC], b16)
            shard_b = spool.tile([P, cells, HIDDEN], b16)
            stats = spool.tile([HIDDEN, 8], f32)  # sum, sumsq, mean, ex2, var, rstd, scale, shift
            NSQ = (NPC + 1023) // 1024
            sqp = spool.tile([HIDDEN, NSQ], f32)
            sq = spool.tile([HIDDEN, 1024], b16)

            for layer in range(4):
                # ---------- table phase: shard tiles + AllGather ----------
                for t in range(cells):
                    pt = ppool2.tile([P, HIDDEN], f32, tag="aux")
                    if layer == 0:
                        xt = tpool.tile([IN_DIM, P], f32, tag="xt")
                        nc.sync.dma_start(out=xt[:], in_=xT_in[:, t * P:(t + 1) * P])
                        nc.tensor.matmul(pt[:], lhsT=xt[:], rhs=w0[:], start=True, stop=True)
                    else:
                        nc.tensor.matmul(pt[:], lhsT=HT[:, t * P:(t + 1) * P],
                                         rhs=wls[layer - 1][:], start=True, stop=True)
                    tt = tpool.tile([P, HIDDEN], f32, tag="tt")
                    nc.vector.tensor_copy(out=tt[:], in_=pt[:])
                    nc.vector.tensor_copy(out=shard_b[:, t, :], in_=pt[:])
                    hi = min(P, n_shard - t * P)
                    nc.sync.dma_start(out=ag_in[t * P:t * P + hi, :], in_=tt[:hi, :])
                nc.gpsimd.collective_compute(
                    "AllGather", mybir.AluOpType.bypass,
                    ins=[ag_in.opt()], outs=[table.opt()],
                    replica_groups=[list(range(ncores))],
                )

                # ---------- self-loop diagonal pass (covers/initializes all ZT) ----------
                for q in range(cells):
                    zq = ppool.tile([HIDDEN, P], f32, tag="zc", name=f"zd_{layer}_{q}")
                    nc.tensor.matmul(zq[:], lhsT=shard_b[:, q, :], rhs=d2[:, q, :],
                                     start=True, stop=True)
                    nc.vector.tensor_copy(out=ZT[:, q * P:(q + 1) * P], in_=zq[:])

                # ---------- gather + scatter ----------
                for k in range(NCHUNK):
                    rb_slots = region_base[k]
                    ncalls = region_slots_pad[k] // SLAB
                    for j in range(ncalls):
                        s0 = rb_slots + j * SLAB          # first slot of call
                        t0 = s0 // P                       # first tile index
                        col0 = s0 // 16                    # idx col base
                        mt = mpool.tile([P, SLAB_T, HIDDEN], f32, tag="m")
                        nc.gpsimd.dma_gather(
                            out_ap=mt[:],
                            in_ap=table[k * chunk_rows:(k + 1) * chunk_rows, :],
                            idxs_ap=idxs[:, col0:col0 + SLAB // 16],
                            num_idxs=SLAB,
                            num_idxs_reg=SLAB,
                            elem_size=HIDDEN,
                            queue_num=(k * 1000 + j) % 2,
                        )
                        mb = mpool.tile([P, SLAB_T, HIDDEN], b16, tag="mb")
                        nc.vector.tensor_tensor(
                            out=mb[:], in0=mt[:],
                            in1=nrm[:, t0:t0 + SLAB_T].rearrange("p (t o) -> p t o", o=1).to_broadcast([P, SLAB_T, HIDDEN]),
                            op=mult_,
                        )
                        st = sbpool.tile([P, SLAB_T, P], b16, tag="s")
                        nc.vector.tensor_tensor(
                            out=st[:],
                            in0=dstl[:, t0:t0 + SLAB_T].rearrange("p (t o) -> p t o", o=1).to_broadcast([P, SLAB_T, P]),
                            in1=iota[:].rearrange("p (o f) -> p o f", o=1).to_broadcast([P, SLAB_T, P]),
                            op=iseq_,
                        )
                        for ti in range(SLAB_T):
                            w0 = tile_win[t0 + ti] if t0 + ti < T_total else None
                            if w0 is None:
                                continue
                            zc = ppool.tile([HIDDEN, P], f32, tag="zc",
                                            name=f"zc_{layer}_{k}_{j}_{ti}")
                            nc.tensor.matmul(zc[:], lhsT=mb[:, ti, :], rhs=st[:, ti, :],
                                             start=True, stop=True)
                            nc.vector.tensor_tensor(out=ZT[:, w0:w0 + P],
                                                    in0=ZT[:, w0:w0 + P],
                                                    in1=zc[:], op=add_)

                # ---------- BatchNorm + ReLU ----------
                nc.vector.reduce_sum(stats[:, 0:1], ZT[:], axis=mybir.AxisListType.X)
                for sc in range(NSQ):
                    w = min(1024, NPC - sc * 1024)
                    nc.vector.tensor_tensor(out=sq[:, :w], in0=ZT[:, sc * 1024:sc * 1024 + w],
                                            in1=ZT[:, sc * 1024:sc * 1024 + w], op=mult_)
                    nc.vector.reduce_sum(sqp[:, sc:sc + 1], sq[:, :w], axis=mybir.AxisListType.X)
                nc.vector.reduce_sum(stats[:, 1:2], sqp[:], axis=mybir.AxisListType.X)
                bt = tpool.tile([HIDDEN, 2], f32, tag="bt")
                nc.vector.tensor_copy(out=bt[:], in_=stats[:, 0:2])
                nc.sync.dma_start(out=bn_in[:], in_=bt[:])
                nc.gpsimd.collective_compute(
                    "AllReduce", add_,
                    ins=[bn_in.opt()], outs=[bn_out.opt()],
                    replica_groups=[list(range(ncores))],
                )
                nc.sync.dma_start(out=stats[:, 0:2], in_=bn_out[:])
                inv_n = 1.0 / float(n_nodes)
                nc.vector.tensor_scalar_mul(stats[:, 2:3], stats[:, 0:1], inv_n)   # mean
                nc.vector.tensor_scalar_mul(stats[:, 3:4], stats[:, 1:2], inv_n)   # E[x^2]
                nc.vector.tensor_tensor(out=stats[:, 4:5], in0=stats[:, 2:3],
                                        in1=stats[:, 2:3], op=mult_)               # mean^2
                nc.vector.tensor_tensor(out=stats[:, 4:5], in0=stats[:, 3:4],
                                        in1=stats[:, 4:5], op=subtract_)           # var
                nc.vector.tensor_scalar_add(stats[:, 4:5], stats[:, 4:5], EPS)
                nc.scalar.activation(stats[:, 5:6], stats[:, 4:5],
                                     mybir.ActivationFunctionType.Sqrt)             # std
                nc.vector.reciprocal(stats[:, 5:6], stats[:, 5:6])                  # rstd
                nc.vector.tensor_tensor(out=stats[:, 6:7], in0=gb[:, layer:layer + 1],
                                        in1=stats[:, 5:6], op=mult_)               # scale
                nc.vector.tensor_tensor(out=stats[:, 7:8], in0=stats[:, 2:3],
                                        in1=stats[:, 6:7], op=mult_)               # mean*scale
                nc.vector.tensor_tensor(out=stats[:, 7:8], in0=gb[:, 4 + layer:5 + layer],
                                        in1=stats[:, 7:8], op=subtract_)           # shift
                nc.vector.scalar_tensor_tensor(
                    out=HT[:], in0=ZT[:], scalar=stats[:, 6:7],
                    in1=stats[:, 7:8].to_broadcast([HIDDEN, NPC]),
                    op0=mult_, op1=add_,
                )
                nc.vector.tensor_scalar(HT[:], HT[:], 0.0, None, op0=max_)

            # ---------- head ----------
            for t in range(cells):
                g = ppool2.tile([HIDDEN, P], f32, tag="aux")
                nc.tensor.matmul(g[:], lhsT=hw1[:], rhs=HT[:, t * P:(t + 1) * P],
                                 start=True, stop=True)
                gs = tpool.tile([HIDDEN, P], b16, tag="gs")
                nc.vector.tensor_tensor(out=gs[:], in0=g[:],
                                        in1=hb1[:].to_broadcast([HIDDEN, P]), op=add_)
                nc.vector.tensor_scalar(gs[:], gs[:], 0.0, None, op0=max_)
                o = ppool2.tile([1, P], f32, tag="aux")
                nc.tensor.matmul(o[:], lhsT=hw2[:], rhs=gs[:], start=True, stop=True)
                ot = tpool.tile([1, P], f32, tag="ot")
                nc.vector.tensor_scalar(ot[:], o[:], hb2, None, op0=add_)
                hi = min(P, n_shard - t * P)
                nc.sync.dma_start(out=y_out[t * P:t * P + hi, :].rearrange("n o -> o n"),
                                  in_=ot[:, :hi])

    nc.finalize()
    _split_big_waits(nc)
    return nc


_CACHE = {}


def kernel(x, edge_index, W0, b0, Ws, bs, gammas, betas, hW1, hb1, hW2, hb2):
    sys.path.insert(0, os.path.dirname(os.path.abspath(__file__)))
    from concourse.bass_utils import run_bass_kernel_spmd

    in_maps, meta = host_prep(x, edge_index, W0, b0, Ws, bs, gammas, betas,
                              hW1, hb1, hW2, hb2)
    key = meta["S_total"]
    if key not in _CACHE:
        _CACHE[key] = build_program(meta)
    nc = _CACHE[key]

    trace = os.environ.get("BASS_GCN_TRACE", "0") == "1"
    kwargs = {}
    if trace:
        import tempfile
        _enable_axon_trace()
        kwargs = dict(trace=True, tmpdir=tempfile.mkdtemp())
    res = run_bass_kernel_spmd(nc, in_maps, core_ids=list(range(NCORES)), **kwargs)
    if trace:
        kernel.last_exec_time_ns = res.exec_time_ns
        kernel.last_trace_dir = kwargs["tmpdir"]
    out = np.concatenate([res.results[c]["y"] for c in range(NCORES)], axis=0)
    return out.astype(np.float32)


# revision 12
# speedup vs baseline: 1.3196x; 1.0017x over previous
"""GCN (4-layer GCNConv + BatchNorm + ReLU, MLP head) on 8 Trainium2 NeuronCores.

Strategy (see DESIGN.md):
- dst-shard nodes across 8 cores (12500 each); weights replicated.
- Per layer: table = H @ W for all nodes, assembled via AllGather of per-core
  shards; per-edge messages fetched with gpsimd.dma_gather (1024-idx calls,
  4x25000-row table chunks for int16 locals); scaled by edge norm on DVE
  (fp32->bf16); segment-sum by destination via one-hot scatter-matmuls into
  PSUM (S built on DVE by iota-compare); self-loops via per-cell diagonal
  matmuls. BatchNorm stats via free-axis reduction + 8-core AllReduce.
- Z/H kept feature-major ([64, nodes]) in SBUF so BN is per-partition.
"""

import os
import sys
import types

sys.path.insert(0, "/opt/trn_rl_repo")
if "/root/.axon_site" not in sys.path:
    sys.path.insert(0, "/root/.axon_site")

import numpy as np
import ml_dtypes

bf16 = ml_dtypes.bfloat16

def _split_big_waits(nc, limit=1):
    """walrus accepts only one sync-wait per instruction; move overflow waits
    onto preceding EventSemaphore ops on the same engine."""
    import concourse.mybir as mybir
    n_split = 0
    for blk in nc.main_func.blocks:
        i = 0
        while i < len(blk.instructions):
            inst = blk.instructions[i]
            si = inst.sync_info
            if si is not None and si.on_wait and len(si.on_wait) > limit:
                waits = list(si.on_wait)
                overflow, keep = waits[:-limit], waits[-limit:]
                idx = i
                for j in range(0, len(overflow), limit):
                    chunk = overflow[j:j + limit]
                    nop = mybir.InstEventSemaphore(
                        name=nc.get_next_instruction_name(), ins=[], outs=[])
                    nop.engine = inst.engine
                    nop.sync_info = mybir.SyncInfo(on_wait=chunk, on_update=[])
                    nc.register_instruction(nop)
                    blk.instructions.insert(idx, nop)
                    idx += 1
                    i += 1
                si.on_wait = keep
                n_split += 1
            i += 1
    return n_split


def _enable_axon_trace():
    """Register the antenv.axon_hooks NTFF shim (profiling under axon)."""
    import antenv
    if "antenv.axon_hooks" in sys.modules:
        return
    mod = types.ModuleType("antenv.axon_hooks")
    _hook = {"h": None}
    mod.set_axon_ntff_profile_hook = lambda h: _hook.__setitem__("h", h)
    mod.get_axon_ntff_profile_hook = lambda: _hook["h"]
    sys.modules["antenv.axon_hooks"] = mod
    antenv.axon_hooks = mod
    from trn_agent_boot.trn_boot import _ntff_profile_via_ctypes
    h = _ntff_profile_via_ctypes("/opt/axon/libaxon_pjrt.so")
    assert h is not None
    mod.set_axon_ntff_profile_hook(h)
    import concourse.bass_utils as bass_utils
    bass_utils.upload_artifacts = lambda tmpdir: str(tmpdir)


N_NODES = 100000
N_EDGES = 1600000
IN_DIM = 128
HIDDEN = 64
EPS = 1e-5
NCORES = 8
P = 128
NCHUNK = 4
SLAB = 1024          # idx per dma_gather call (HW ring limit ~1024)
SLAB_T = SLAB // P   # 8 tiles per slab


def _wrap_idx16(idx):
    """[n] -> [128, n//16] int16 wrapped in 16 partitions, replicated x8."""
    n16 = len(idx) // 16
    w = np.asarray(idx, np.int16).reshape(n16, 16).T
    return np.tile(w, (8, 1))


def host_prep(x, edge_index, W0, b0, Ws, bs, gammas, betas, hW1, hb1, hW2, hb2,
              n_nodes=N_NODES, ncores=NCORES):
    """Shard + preprocess. Returns (in_maps, meta) for the SPMD program."""
    n_shard = n_nodes // ncores
    cells = (n_shard + P - 1) // P            # dst cells per core
    chunk_rows = n_nodes // NCHUNK            # table chunk height

    src = np.asarray(edge_index[0], np.int64)
    dst = np.asarray(edge_index[1], np.int64)
    deg = np.bincount(dst, minlength=n_nodes).astype(np.float64) + 1.0  # + self loop
    dinv = (1.0 / np.sqrt(deg)).astype(np.float64)
    dinv2 = (dinv * dinv).astype(np.float32)

    core_of = dst // n_shard
    chunk_of = src // chunk_rows

    BLK = 64                                   # dsts per budget block
    nblk = (n_shard + BLK - 1) // BLK          # blocks per core (per chunk)
    dstl_all = dst - core_of * n_shard
    blk_of = dstl_all // BLK
    counts = np.zeros((ncores, NCHUNK, nblk), np.int64)
    np.add.at(counts, (core_of, chunk_of, blk_of), 1)
    budget = np.maximum(counts.max(axis=0), P)  # [NCHUNK, nblk] slots, >= 128

    # slot layout: chunk-major regions; block b gets budget[k, b] slots
    region_slots = budget.sum(axis=1)
    region_slots_pad = -(-region_slots // SLAB) * SLAB
    S_total = int(region_slots_pad.sum())
    T_total = S_total // P

    blk_base = np.zeros((NCHUNK, nblk), np.int64)
    base = 0
    region_base = []
    for k in range(NCHUNK):
        region_base.append(base)
        for b in range(nblk):
            blk_base[k, b] = base
            base += int(budget[k, b])
        base = region_base[k] + int(region_slots_pad[k])
    assert base == S_total

    # per-tile dst window (uniform across cores): win0 = BLK * block(first slot),
    # clamped; None for region-tail tiles holding no block slots
    cells = (n_shard + P - 1) // P
    NPC = cells * P
    slot_block = np.full(S_total, -1, np.int64)   # block of each slot (k-local)
    for k in range(NCHUNK):
        for b in range(nblk):
            s0 = int(blk_base[k, b])
            slot_block[s0:s0 + int(budget[k, b])] = b
    tile_win = []
    region_of_tile = []
    for t in range(T_total):
        b = slot_block[t * P]
        tile_win.append(None if b < 0 else int(min(b * BLK, NPC - P)))
        # which chunk region this tile is in
        k_of = 0
        for k in range(NCHUNK):
            if t * P >= region_base[k]:
                k_of = k
        region_of_tile.append(k_of)
    # chains: consecutive tiles, same window, same region
    tile_flags = []  # (is_first, is_last) per tile (None for tail tiles)
    for t in range(T_total):
        if tile_win[t] is None:
            tile_flags.append(None)
            continue
        prev_same = (t > 0 and tile_win[t - 1] == tile_win[t]
                     and region_of_tile[t - 1] == region_of_tile[t])
        next_same = (t + 1 < T_total and tile_win[t + 1] == tile_win[t]
                     and region_of_tile[t + 1] == region_of_tile[t])
        tile_flags.append((not prev_same, not next_same))

    in_maps = []
    for c in range(ncores):
        m = core_of == c
        src_c = src[m]
        dstl_c = dstl_all[m]
        k_c = chunk_of[m]
        b_c = blk_of[m]
        norm_c = (dinv[src_c] * dinv[dst[m]]).astype(np.float32)

        order = np.lexsort((dstl_c, b_c, k_c))
        src_c, dstl_c, k_c, b_c, norm_c = (a[order] for a in (src_c, dstl_c, k_c, b_c, norm_c))

        idx_slots = np.zeros(S_total, np.int16)
        dl_slots = np.full(S_total, 255.0, np.float32)
        nm_slots = np.zeros(S_total, np.float32)
        ptr = 0
        for k in range(NCHUNK):
            cnts = counts[c, k]
            for b in range(nblk):
                n_kb = int(cnts[b])
                sl = slice(ptr, ptr + n_kb)
                s0 = int(blk_base[k, b])
                idx_slots[s0:s0 + n_kb] = (src_c[sl] - k * chunk_rows).astype(np.int16)
                # dst_local relative to the WINDOW of each slot's tile
                ss = np.arange(s0, s0 + n_kb)
                w0 = np.array([tile_win[t] for t in ss // P], np.int64)
                dl_slots[s0:s0 + n_kb] = (dstl_c[sl] - w0).astype(np.float32)
                nm_slots[s0:s0 + n_kb] = norm_c[sl]
                ptr += n_kb
        assert ptr == len(src_c)
        assert (dl_slots >= 0).all() and (dl_slots <= 255).all()

        # slot s -> [s % 128, s // 128]
        idxw = np.concatenate(
            [_wrap_idx16(idx_slots[region_base[k]:region_base[k] + int(region_slots_pad[k])])
             for k in range(NCHUNK)], axis=1)
        # NOTE: wrapped layout is per-16 within each slab of its region; dma_gather
        # unwraps [16, :] as (s p): slot i of a call -> idx at [i % 16, i // 16] of the
        # call's column slice. Building per-region keeps call slices aligned.
        dl_arr = dl_slots.reshape(T_total, P).T.astype(bf16)      # [128, T]
        nm_arr = nm_slots.reshape(T_total, P).T.astype(np.float32)  # [128, T] fp32

        # D2: [128, cells*128] bf16; [r, q*128 + j] = (r == j) * dinv2[shard node q*128+r]
        d2 = np.zeros((P, cells * P), np.float32)
        for q in range(cells):
            nmax = min(P, n_shard - q * P)
            rr = np.arange(nmax)
            d2[rr, q * P + rr] = dinv2[c * n_shard + q * P + rr]
        # x shard transposed, padded to cells*128 columns
        xT = np.zeros((IN_DIM, cells * P), np.float32)
        xT[:, :n_shard] = np.asarray(x, np.float32)[c * n_shard:(c + 1) * n_shard].T

        in_maps.append({
            "xT": xT,
            "idxw": np.ascontiguousarray(idxw),
            "dstl": np.ascontiguousarray(dl_arr),
            "nrm": np.ascontiguousarray(nm_arr),
            "d2": d2.astype(bf16),
            "w0": np.asarray(W0, np.float32),
            "wl": np.asarray(Ws, np.float32).astype(bf16),
            "gb": np.concatenate([np.asarray(gammas, np.float32).T,
                                  np.asarray(betas, np.float32).T], axis=1),  # [64, 8]
            "hw1": np.asarray(hW1, np.float32).astype(bf16),
            "hb1": np.asarray(hb1, np.float32).reshape(HIDDEN, 1),
            "hw2": np.asarray(hW2, np.float32).astype(bf16).reshape(HIDDEN, 1),
            "iota": np.tile(np.arange(P, dtype=np.float32).reshape(1, P), (P, 1)).astype(bf16),
        })

    meta = dict(n_nodes=n_nodes, n_shard=n_shard, cells=cells, chunk_rows=chunk_rows,
                S_total=S_total, T_total=T_total, tile_win=tile_win,
                tile_flags=tile_flags,
                region_base=region_base,
                region_slots_pad=[int(v) for v in region_slots_pad],
                hb2=float(np.asarray(hb2).reshape(-1)[0]))
    return in_maps, meta


def build_program(meta, ncores=NCORES):
    import concourse.bass as bass
    import concourse.bacc as bacc
    import concourse.mybir as mybir
    import concourse.tile as tile

    n_nodes = meta["n_nodes"]
    n_shard = meta["n_shard"]
    cells = meta["cells"]
    chunk_rows = meta["chunk_rows"]
    S_total = meta["S_total"]
    T_total = meta["T_total"]
    tile_win = meta["tile_win"]
    tile_flags = meta["tile_flags"]
    region_base = meta["region_base"]
    region_slots_pad = meta["region_slots_pad"]
    hb2 = meta["hb2"]
    NPC = cells * P  # padded shard width

    f32 = mybir.dt.float32
    b16 = mybir.dt.bfloat16
    add_ = mybir.AluOpType.add
    mult_ = mybir.AluOpType.mult
    iseq_ = mybir.AluOpType.is_equal
    max_ = mybir.AluOpType.max
    subtract_ = mybir.AluOpType.subtract

    nc = bacc.Bacc(num_devices=ncores, num_swdge_queues=2)
    xT_in = nc.declare_dram_parameter("xT", [IN_DIM, NPC], f32, isOutput=False)
    idx_in = nc.declare_dram_parameter("idxw", [P, S_total // 16], mybir.dt.int16, isOutput=False)
    dstl_in = nc.declare_dram_parameter("dstl", [P, T_total], b16, isOutput=False)
    nrm_in = nc.declare_dram_parameter("nrm", [P, T_total], f32, isOutput=False)
    d2_in = nc.declare_dram_parameter("d2", [P, cells * P], b16, isOutput=False)
    w0_in = nc.declare_dram_parameter("w0", [IN_DIM, HIDDEN], f32, isOutput=False)
    wl_in = nc.declare_dram_parameter("wl", [3, HIDDEN, HIDDEN], b16, isOutput=False)
    gb_in = nc.declare_dram_parameter("gb", [HIDDEN, 8], f32, isOutput=False)
    hw1_in = nc.declare_dram_parameter("hw1", [HIDDEN, HIDDEN], b16, isOutput=False)
    hb1_in = nc.declare_dram_parameter("hb1", [HIDDEN, 1], f32, isOutput=False)
    hw2_in = nc.declare_dram_parameter("hw2", [HIDDEN, 1], b16, isOutput=False)
    iota_in = nc.declare_dram_parameter("iota", [P, P], b16, isOutput=False)
    y_out = nc.declare_dram_parameter("y", [n_shard, 1], f32, isOutput=True)

    with tile.TileContext(nc, num_cores=ncores) as tc:
        with (
            tc.tile_pool(name="dram", bufs=1, space="DRAM") as dpool,
            tc.tile_pool(name="const", bufs=1) as cpool,
            tc.tile_pool(name="state", bufs=1) as spool,
            tc.tile_pool(name="mslab", bufs=6) as mpool,
            tc.tile_pool(name="sslab", bufs=6) as sbpool,
            tc.tile_pool(name="ttile", bufs=4) as tpool,
            tc.tile_pool(name="psum", bufs=4, space="PSUM") as ppool,
            tc.tile_pool(name="psum2", bufs=2, space="PSUM") as ppool2,
        ):
            ag_in = dpool.tile([n_shard, HIDDEN], f32)
            table = dpool.tile([n_nodes, HIDDEN], f32)
            bn_in = dpool.tile([HIDDEN, 2], f32)
            bn_out = dpool.tile([HIDDEN, 2], f32)

            # ---- constants to SBUF ----
            idxs = cpool.tile([P, S_total // 16], mybir.dt.int16)
            nc.sync.dma_start(out=idxs[:], in_=idx_in[:])
            dstl = cpool.tile([P, T_total], b16)
            nc.sync.dma_start(out=dstl[:], in_=dstl_in[:])
            nrm = cpool.tile([P, T_total], f32)
            nc.sync.dma_start(out=nrm[:], in_=nrm_in[:])
            d2 = cpool.tile([P, cells, P], b16)
            nc.sync.dma_start(out=d2[:], in_=d2_in.rearrange("p (q j) -> p q j", j=P))
            w0 = cpool.tile([IN_DIM, HIDDEN], f32)
            nc.sync.dma_start(out=w0[:], in_=w0_in[:])
            wls = []
            for i in range(3):
                wli = cpool.tile([HIDDEN, HIDDEN], b16, name=f"wl{i}")
                nc.sync.dma_start(out=wli[:], in_=wl_in[i])
                wls.append(wli)
            gb = cpool.tile([HIDDEN, 8], f32)
            nc.sync.dma_start(out=gb[:], in_=gb_in[:])
            hw1 = cpool.tile([HIDDEN, HIDDEN], b16)
            nc.sync.dma_start(out=hw1[:], in_=hw1_in[:])
            hb1 = cpool.tile([HIDDEN, 1], f32)
            nc.sync.dma_start(out=hb1[:], in_=hb1_in[:])
            hw2 = cpool.tile([HIDDEN, 1], b16)
            nc.sync.dma_start(out=hw2[:], in_=hw2_in[:])
            iota = cpool.tile([P, P], b16)
            nc.sync.dma_start(out=iota[:], in_=iota_in[:])

            # ---- state ----
            ZT = spool.tile([HIDDEN, NPC], f32)
            HT = spool.tile([HIDDEN, NPC], b16)
            shard_b = spool.tile([P, cells, HIDDEN], b16)
            stats = spool.tile([HIDDEN, 8], f32)  # sum, sumsq, mean, ex2, var, rstd, scale, shift
            NSQ = (NPC + 1023) // 1024
            sqp = spool.tile([HIDDEN, NSQ], f32)
            sq = spool.tile([HIDDEN, 1024], b16)

            for layer in range(4):
                # ---------- table phase: shard tiles + AllGather ----------
                for t in range(cells):
                    pt = ppool2.tile([P, HIDDEN], f32, tag="aux")
                    if layer == 0:
                        xt = tpool.tile([IN_DIM, P], f32, tag="xt")
                        nc.sync.dma_start(out=xt[:], in_=xT_in[:, t * P:(t + 1) * P])
                        nc.tensor.matmul(pt[:], lhsT=xt[:], rhs=w0[:], start=True, stop=True)
                    else:
                        nc.tensor.matmul(pt[:], lhsT=HT[:, t * P:(t + 1) * P],
                                         rhs=wls[layer - 1][:], start=True, stop=True)
                    tt = tpool.tile([P, HIDDEN], f32, tag="tt")
                    nc.vector.tensor_copy(out=tt[:], in_=pt[:])
                    nc.vector.tensor_copy(out=shard_b[:, t, :], in_=pt[:])
                    hi = min(P, n_shard - t * P)
                    nc.sync.dma_start(out=ag_in[t * P:t * P + hi, :], in_=tt[:hi, :])
                nc.gpsimd.collective_compute(
                    "AllGather", mybir.AluOpType.bypass,
                    ins=[ag_in.opt()], outs=[table.opt()],
                    replica_groups=[list(range(ncores))],
                )

                # ---------- self-loop diagonal pass (covers/initializes all ZT) ----------
                for q in range(cells):
                    zq = ppool.tile([HIDDEN, P], f32, tag="zc", name=f"zd_{layer}_{q}")
                    nc.tensor.matmul(zq[:], lhsT=shard_b[:, q, :], rhs=d2[:, q, :],
                                     start=True, stop=True)
                    nc.vector.tensor_copy(out=ZT[:, q * P:(q + 1) * P], in_=zq[:])

                # ---------- gather + scatter ----------
                for k in range(NCHUNK):
                    rb_slots = region_base[k]
                    ncalls = region_slots_pad[k] // SLAB
                    for j in range(ncalls):
                        s0 = rb_slots + j * SLAB          # first slot of call
                        t0 = s0 // P                       # first tile index
                        col0 = s0 // 16                    # idx col base
                        mt = mpool.tile([P, SLAB_T, HIDDEN], f32, tag="m")
                        nc.gpsimd.dma_gather(
                            out_ap=mt[:],
                            in_ap=table[k * chunk_rows:(k + 1) * chunk_rows, :],
                            idxs_ap=idxs[:, col0:col0 + SLAB // 16],
                            num_idxs=SLAB,
                            num_idxs_reg=SLAB,
                            elem_size=HIDDEN,
                            queue_num=(k * 1000 + j) % 2,
                        )
                        mb = mpool.tile([P, SLAB_T, HIDDEN], b16, tag="mb")
                        nc.vector.tensor_tensor(
                            out=mb[:], in0=mt[:],
                            in1=nrm[:, t0:t0 + SLAB_T].rearrange("p (t o) -> p t o", o=1).to_broadcast([P, SLAB_T, HIDDEN]),
                            op=mult_,
                        )
                        st = sbpool.tile([P, SLAB_T, P], b16, tag="s")
                        nc.vector.tensor_tensor(
                            out=st[:],
                            in0=dstl[:, t0:t0 + SLAB_T].rearrange("p (t o) -> p t o", o=1).to_broadcast([P, SLAB_T, P]),
                            in1=iota[:].rearrange("p (o f) -> p o f", o=1).to_broadcast([P, SLAB_T, P]),
                            op=iseq_,
                        )
                        for ti in range(SLAB_T):
                            tt_ = t0 + ti
                            w0 = tile_win[tt_] if tt_ < T_total else None
                            if w0 is None:
                                continue
                            is_first, is_last = tile_flags[tt_]
                            if is_first:
                                cur_zc = ppool.tile([HIDDEN, P], f32, tag="zc",
                                                    name=f"zc_{layer}_{k}_{j}_{ti}")
                            nc.tensor.matmul(cur_zc[:], lhsT=mb[:, ti, :], rhs=st[:, ti, :],
                                             start=is_first, stop=is_last)
                            if is_last:
                                nc.vector.tensor_tensor(out=ZT[:, w0:w0 + P],
                                                        in0=ZT[:, w0:w0 + P],
                                                        in1=cur_zc[:], op=add_)

                # ---------- BatchNorm + ReLU ----------
                nc.vector.reduce_sum(stats[:, 0:1], ZT[:], axis=mybir.AxisListType.X)
                for sc in range(NSQ):
                    w = min(1024, NPC - sc * 1024)
                    nc.vector.tensor_tensor(out=sq[:, :w], in0=ZT[:, sc * 1024:sc * 1024 + w],
                                            in1=ZT[:, sc * 1024:sc * 1024 + w], op=mult_)
                    nc.vector.reduce_sum(sqp[:, sc:sc + 1], sq[:, :w], axis=mybir.AxisListType.X)
                nc.vector.reduce_sum(stats[:, 1:2], sqp[:], axis=mybir.AxisListType.X)
                bt = tpool.tile([HIDDEN, 2], f32, tag="bt")
                nc.vector.tensor_copy(out=bt[:], in_=stats[:, 0:2])
                nc.sync.dma_start(out=bn_in[:], in_=bt[:])
                nc.gpsimd.collective_compute(
                    "AllReduce", add_,
                    ins=[bn_in.opt()], outs=[bn_out.opt()],
                    replica_groups=[list(range(ncores))],
                )
                nc.sync.dma_start(out=stats[:, 0:2], in_=bn_out[:])
                inv_n = 1.0 / float(n_nodes)
                nc.vector.tensor_scalar_mul(stats[:, 2:3], stats[:, 0:1], inv_n)   # mean
                nc.vector.tensor_scalar_mul(stats[:, 3:4], stats[:, 1:2], inv_n)   # E[x^2]
                nc.vector.tensor_tensor(out=stats[:, 4:5], in0=stats[:, 2:3],
                                        in1=stats[:, 2:3], op=mult_)               # mean^2
                nc.vector.tensor_tensor(out=stats[:, 4:5], in0=stats[:, 3:4],
                                        in1=stats[:, 4:5], op=subtract_)           # var
                nc.vector.tensor_scalar_add(stats[:, 4:5], stats[:, 4:5], EPS)
                nc.scalar.activation(stats[:, 5:6], stats[:, 4:5],
                                     mybir.ActivationFunctionType.Sqrt)             # std
                nc.vector.reciprocal(stats[:, 5:6], stats[:, 5:6])                  # rstd
                nc.vector.tensor_tensor(out=stats[:, 6:7], in0=gb[:, layer:layer + 1],
                                        in1=stats[:, 5:6], op=mult_)               # scale
                nc.vector.tensor_tensor(out=stats[:, 7:8], in0=stats[:, 2:3],
                                        in1=stats[:, 6:7], op=mult_)               # mean*scale
                nc.vector.tensor_tensor(out=stats[:, 7:8], in0=gb[:, 4 + layer:5 + layer],
                                        in1=stats[:, 7:8], op=subtract_)           # shift
                nc.vector.scalar_tensor_tensor(
                    out=HT[:], in0=ZT[:], scalar=stats[:, 6:7],
                    in1=stats[:, 7:8].to_broadcast([HIDDEN, NPC]),
                    op0=mult_, op1=add_,
                )
                nc.vector.tensor_scalar(HT[:], HT[:], 0.0, None, op0=max_)

            # ---------- head ----------
            for t in range(cells):
                g = ppool2.tile([HIDDEN, P], f32, tag="aux")
                nc.tensor.matmul(g[:], lhsT=hw1[:], rhs=HT[:, t * P:(t + 1) * P],
                                 start=True, stop=True)
                gs = tpool.tile([HIDDEN, P], b16, tag="gs")
                nc.vector.tensor_tensor(out=gs[:], in0=g[:],
                                        in1=hb1[:].to_broadcast([HIDDEN, P]), op=add_)
                nc.vector.tensor_scalar(gs[:], gs[:], 0.0, None, op0=max_)
                o = ppool2.tile([1, P], f32, tag="aux")
                nc.tensor.matmul(o[:], lhsT=hw2[:], rhs=gs[:], start=True, stop=True)
                ot = tpool.tile([1, P], f32, tag="ot")
                nc.vector.tensor_scalar(ot[:], o[:], hb2, None, op0=add_)
                hi = min(P, n_shard - t * P)
                nc.sync.dma_start(out=y_out[t * P:t * P + hi, :].rearrange("n o -> o n"),
                                  in_=ot[:, :hi])

    nc.finalize()
    _split_big_waits(nc)
    return nc


_CACHE = {}


def kernel(x, edge_index, W0, b0, Ws, bs, gammas, betas, hW1, hb1, hW2, hb2):
    sys.path.insert(0, os.path.dirname(os.path.abspath(__file__)))
    from concourse.bass_utils import run_bass_kernel_spmd

    in_maps, meta = host_prep(x, edge_index, W0, b0, Ws, bs, gammas, betas,
                              hW1, hb1, hW2, hb2)
    key = meta["S_total"]
    if key not in _CACHE:
        _CACHE[key] = build_program(meta)
    nc = _CACHE[key]

    trace = os.environ.get("BASS_GCN_TRACE", "0") == "1"
    kwargs = {}
    if trace:
        import tempfile
        _enable_axon_trace()
        kwargs = dict(trace=True, tmpdir=tempfile.mkdtemp())
    res = run_bass_kernel_spmd(nc, in_maps, core_ids=list(range(NCORES)), **kwargs)
    if trace:
        kernel.last_exec_time_ns = res.exec_time_ns
        kernel.last_trace_dir = kwargs["tmpdir"]
    out = np.concatenate([res.results[c]["y"] for c in range(NCORES)], axis=0)
    return out.astype(np.float32)


# revision 16
# speedup vs baseline: 1.4793x; 1.1210x over previous
"""GCN (4-layer GCNConv + BatchNorm + ReLU, MLP head) on 8 Trainium2 NeuronCores.

Strategy (see DESIGN.md):
- dst-shard nodes across 8 cores (12500 each); weights replicated.
- Per layer: table = H @ W for all nodes, assembled via AllGather of per-core
  shards; per-edge messages fetched with gpsimd.dma_gather (1024-idx calls,
  4x25000-row table chunks for int16 locals); scaled by edge norm on DVE
  (fp32->bf16); segment-sum by destination via one-hot scatter-matmuls into
  PSUM (S built on DVE by iota-compare); self-loops via per-cell diagonal
  matmuls. BatchNorm stats via free-axis reduction + 8-core AllReduce.
- Z/H kept feature-major ([64, nodes]) in SBUF so BN is per-partition.
"""

import os
import sys
import types

sys.path.insert(0, "/opt/trn_rl_repo")
if "/root/.axon_site" not in sys.path:
    sys.path.insert(0, "/root/.axon_site")

import numpy as np
import ml_dtypes

bf16 = ml_dtypes.bfloat16

def _split_big_waits(nc, limit=1):
    """walrus accepts only one sync-wait per instruction; move overflow waits
    onto preceding EventSemaphore ops on the same engine."""
    import concourse.mybir as mybir
    n_split = 0
    for blk in nc.main_func.blocks:
        i = 0
        while i < len(blk.instructions):
            inst = blk.instructions[i]
            si = inst.sync_info
            if si is not None and si.on_wait and len(si.on_wait) > limit:
                waits = list(si.on_wait)
                overflow, keep = waits[:-limit], waits[-limit:]
                idx = i
                for j in range(0, len(overflow), limit):
                    chunk = overflow[j:j + limit]
                    nop = mybir.InstEventSemaphore(
                        name=nc.get_next_instruction_name(), ins=[], outs=[])
                    nop.engine = inst.engine
                    nop.sync_info = mybir.SyncInfo(on_wait=chunk, on_update=[])
                    nc.register_instruction(nop)
                    blk.instructions.insert(idx, nop)
                    idx += 1
                    i += 1
                si.on_wait = keep
                n_split += 1
            i += 1
    return n_split


def _enable_axon_trace():
    """Register the antenv.axon_hooks NTFF shim (profiling under axon)."""
    import antenv
    if "antenv.axon_hooks" in sys.modules:
        return
    mod = types.ModuleType("antenv.axon_hooks")
    _hook = {"h": None}
    mod.set_axon_ntff_profile_hook = lambda h: _hook.__setitem__("h", h)
    mod.get_axon_ntff_profile_hook = lambda: _hook["h"]
    sys.modules["antenv.axon_hooks"] = mod
    antenv.axon_hooks = mod
    from trn_agent_boot.trn_boot import _ntff_profile_via_ctypes
    h = _ntff_profile_via_ctypes("/opt/axon/libaxon_pjrt.so")
    assert h is not None
    mod.set_axon_ntff_profile_hook(h)
    import concourse.bass_utils as bass_utils
    bass_utils.upload_artifacts = lambda tmpdir: str(tmpdir)


N_NODES = 100000
N_EDGES = 1600000
IN_DIM = 128
HIDDEN = 64
EPS = 1e-5
NCORES = 8
P = 128
NCHUNK = 4
SLAB = 1024          # idx per dma_gather call (HW ring limit ~1024)
SLAB_T = SLAB // P   # 8 tiles per slab


def _wrap_idx16(idx):
    """[n] -> [128, n//16] int16 wrapped in 16 partitions, replicated x8."""
    n16 = len(idx) // 16
    w = np.asarray(idx, np.int16).reshape(n16, 16).T
    return np.tile(w, (8, 1))


def host_prep(x, edge_index, W0, b0, Ws, bs, gammas, betas, hW1, hb1, hW2, hb2,
              n_nodes=N_NODES, ncores=NCORES):
    """Shard + preprocess. Returns (in_maps, meta) for the SPMD program."""
    n_shard = n_nodes // ncores
    cells = (n_shard + P - 1) // P            # dst cells per core
    # shard quarters (tile-aligned except the last): chunk q of the table is the
    # AllGather of every core's quarter q, laid out rank-major.
    qtiles = [25, 25, 25, cells - 75] if cells == 98 else [cells // 4] * 3 + [cells - 3 * (cells // 4)]
    qsize = []
    qstart = [0]
    for qi in range(NCHUNK):
        hi = min(n_shard, (qstart[-1] + qtiles[qi] * P))
        qsize.append(hi - qstart[-1])
        qstart.append(hi)
    qstart = qstart[:NCHUNK]
    chunk_rows_q = [ncores * s for s in qsize]

    src = np.asarray(edge_index[0], np.int64)
    dst = np.asarray(edge_index[1], np.int64)
    deg = np.bincount(dst, minlength=n_nodes).astype(np.float64) + 1.0  # + self loop
    dinv = (1.0 / np.sqrt(deg)).astype(np.float64)
    dinv2 = (dinv * dinv).astype(np.float32)

    core_of = dst // n_shard
    src_r = src // n_shard
    src_j = src % n_shard
    chunk_of = np.searchsorted(np.array(qstart[1:] + [n_shard]), src_j, side="right")
    # local row of src within its chunk's rank-major table
    qs = np.array(qstart)[chunk_of]
    qz = np.array(qsize)[chunk_of]
    src_local = src_r * qz + (src_j - qs)

    BLK = 64                                   # dsts per budget block
    nblk = (n_shard + BLK - 1) // BLK          # blocks per core (per chunk)
    dstl_all = dst - core_of * n_shard
    blk_of = dstl_all // BLK
    counts = np.zeros((ncores, NCHUNK, nblk), np.int64)
    np.add.at(counts, (core_of, chunk_of, blk_of), 1)
    budget = np.maximum(counts.max(axis=0), P)  # [NCHUNK, nblk] slots, >= 128

    # slot layout: chunk-major regions; block b gets budget[k, b] slots
    region_slots = budget.sum(axis=1)
    region_slots_pad = -(-region_slots // SLAB) * SLAB
    S_total = int(region_slots_pad.sum())
    T_total = S_total // P

    blk_base = np.zeros((NCHUNK, nblk), np.int64)
    base = 0
    region_base = []
    for k in range(NCHUNK):
        region_base.append(base)
        for b in range(nblk):
            blk_base[k, b] = base
            base += int(budget[k, b])
        base = region_base[k] + int(region_slots_pad[k])
    assert base == S_total

    # per-tile dst window (uniform across cores): win0 = BLK * block(first slot),
    # clamped; None for region-tail tiles holding no block slots
    cells = (n_shard + P - 1) // P
    NPC = cells * P
    slot_block = np.full(S_total, -1, np.int64)   # block of each slot (k-local)
    for k in range(NCHUNK):
        for b in range(nblk):
            s0 = int(blk_base[k, b])
            slot_block[s0:s0 + int(budget[k, b])] = b
    tile_win = []
    region_of_tile = []
    for t in range(T_total):
        b = slot_block[t * P]
        tile_win.append(None if b < 0 else int(min(b * BLK, NPC - P)))
        # which chunk region this tile is in
        k_of = 0
        for k in range(NCHUNK):
            if t * P >= region_base[k]:
                k_of = k
        region_of_tile.append(k_of)
    # chains: consecutive tiles, same window, same region
    tile_flags = []  # (is_first, is_last) per tile (None for tail tiles)
    for t in range(T_total):
        if tile_win[t] is None:
            tile_flags.append(None)
            continue
        prev_same = (t > 0 and tile_win[t - 1] == tile_win[t]
                     and region_of_tile[t - 1] == region_of_tile[t])
        next_same = (t + 1 < T_total and tile_win[t + 1] == tile_win[t]
                     and region_of_tile[t + 1] == region_of_tile[t])
        tile_flags.append((not prev_same, not next_same))

    in_maps = []
    for c in range(ncores):
        m = core_of == c
        src_c = src[m]
        srcl_c = src_local[m]
        dstl_c = dstl_all[m]
        k_c = chunk_of[m]
        b_c = blk_of[m]
        norm_c = (dinv[src_c] * dinv[dst[m]]).astype(np.float32)

        order = np.lexsort((dstl_c, b_c, k_c))
        src_c, srcl_c, dstl_c, k_c, b_c, norm_c = (
            a[order] for a in (src_c, srcl_c, dstl_c, k_c, b_c, norm_c))

        idx_slots = np.zeros(S_total, np.int16)
        dl_slots = np.full(S_total, 255.0, np.float32)
        nm_slots = np.zeros(S_total, np.float32)
        ptr = 0
        for k in range(NCHUNK):
            cnts = counts[c, k]
            for b in range(nblk):
                n_kb = int(cnts[b])
                sl = slice(ptr, ptr + n_kb)
                s0 = int(blk_base[k, b])
                idx_slots[s0:s0 + n_kb] = srcl_c[sl].astype(np.int16)
                # dst_local relative to the WINDOW of each slot's tile
                ss = np.arange(s0, s0 + n_kb)
                w0 = np.array([tile_win[t] for t in ss // P], np.int64)
                dl_slots[s0:s0 + n_kb] = (dstl_c[sl] - w0).astype(np.float32)
                nm_slots[s0:s0 + n_kb] = norm_c[sl]
                ptr += n_kb
        assert ptr == len(src_c)
        assert (dl_slots >= 0).all() and (dl_slots <= 255).all()

        # slot s -> [s % 128, s // 128]
        idxw = np.concatenate(
            [_wrap_idx16(idx_slots[region_base[k]:region_base[k] + int(region_slots_pad[k])])
             for k in range(NCHUNK)], axis=1)
        # NOTE: wrapped layout is per-16 within each slab of its region; dma_gather
        # unwraps [16, :] as (s p): slot i of a call -> idx at [i % 16, i // 16] of the
        # call's column slice. Building per-region keeps call slices aligned.
        dl_arr = dl_slots.reshape(T_total, P).T.astype(bf16)      # [128, T]
        nm_arr = nm_slots.reshape(T_total, P).T.astype(np.float32)  # [128, T] fp32

        # D2: [128, cells*128] bf16; [r, q*128 + j] = (r == j) * dinv2[shard node q*128+r]
        d2 = np.zeros((P, cells * P), np.float32)
        for q in range(cells):
            nmax = min(P, n_shard - q * P)
            rr = np.arange(nmax)
            d2[rr, q * P + rr] = dinv2[c * n_shard + q * P + rr]
        # x shard transposed, padded to cells*128 columns
        xT = np.zeros((IN_DIM, cells * P), np.float32)
        xT[:, :n_shard] = np.asarray(x, np.float32)[c * n_shard:(c + 1) * n_shard].T

        in_maps.append({
            "xT": xT,
            "idxw": np.ascontiguousarray(idxw),
            "dstl": np.ascontiguousarray(dl_arr),
            "nrm": np.ascontiguousarray(nm_arr),
            "d2": d2.astype(bf16),
            "w0": np.asarray(W0, np.float32),
            "wl": np.asarray(Ws, np.float32).astype(bf16),
            "gb": np.concatenate([np.asarray(gammas, np.float32).T,
                                  np.asarray(betas, np.float32).T], axis=1),  # [64, 8]
            "hw1": np.asarray(hW1, np.float32).astype(bf16),
            "hb1": np.asarray(hb1, np.float32).reshape(HIDDEN, 1),
            "hw2": np.asarray(hW2, np.float32).astype(bf16).reshape(HIDDEN, 1),
            "iota": np.tile(np.arange(P, dtype=np.float32).reshape(1, P), (P, 1)).astype(bf16),
        })

    meta = dict(n_nodes=n_nodes, n_shard=n_shard, cells=cells,
                S_total=S_total, T_total=T_total, tile_win=tile_win,
                tile_flags=tile_flags, qtiles=qtiles, qsize=qsize, qstart=qstart,
                chunk_rows_q=chunk_rows_q,
                region_base=region_base,
                region_slots_pad=[int(v) for v in region_slots_pad],
                hb2=float(np.asarray(hb2).reshape(-1)[0]))
    return in_maps, meta


def build_program(meta, ncores=NCORES):
    import concourse.bass as bass
    import concourse.bacc as bacc
    import concourse.mybir as mybir
    import concourse.tile as tile

    n_nodes = meta["n_nodes"]
    n_shard = meta["n_shard"]
    cells = meta["cells"]
    qtiles = meta["qtiles"]
    qsize = meta["qsize"]
    qstart = meta["qstart"]
    chunk_rows_q = meta["chunk_rows_q"]
    S_total = meta["S_total"]
    T_total = meta["T_total"]
    tile_win = meta["tile_win"]
    tile_flags = meta["tile_flags"]
    region_base = meta["region_base"]
    region_slots_pad = meta["region_slots_pad"]
    hb2 = meta["hb2"]
    NPC = cells * P  # padded shard width

    f32 = mybir.dt.float32
    b16 = mybir.dt.bfloat16
    add_ = mybir.AluOpType.add
    mult_ = mybir.AluOpType.mult
    iseq_ = mybir.AluOpType.is_equal
    max_ = mybir.AluOpType.max
    subtract_ = mybir.AluOpType.subtract

    nc = bacc.Bacc(num_devices=ncores, num_swdge_queues=2)
    xT_in = nc.declare_dram_parameter("xT", [IN_DIM, NPC], f32, isOutput=False)
    idx_in = nc.declare_dram_parameter("idxw", [P, S_total // 16], mybir.dt.int16, isOutput=False)
    dstl_in = nc.declare_dram_parameter("dstl", [P, T_total], b16, isOutput=False)
    nrm_in = nc.declare_dram_parameter("nrm", [P, T_total], f32, isOutput=False)
    d2_in = nc.declare_dram_parameter("d2", [P, cells * P], b16, isOutput=False)
    w0_in = nc.declare_dram_parameter("w0", [IN_DIM, HIDDEN], f32, isOutput=False)
    wl_in = nc.declare_dram_parameter("wl", [3, HIDDEN, HIDDEN], b16, isOutput=False)
    gb_in = nc.declare_dram_parameter("gb", [HIDDEN, 8], f32, isOutput=False)
    hw1_in = nc.declare_dram_parameter("hw1", [HIDDEN, HIDDEN], b16, isOutput=False)
    hb1_in = nc.declare_dram_parameter("hb1", [HIDDEN, 1], f32, isOutput=False)
    hw2_in = nc.declare_dram_parameter("hw2", [HIDDEN, 1], b16, isOutput=False)
    iota_in = nc.declare_dram_parameter("iota", [P, P], b16, isOutput=False)
    y_out = nc.declare_dram_parameter("y", [n_shard, 1], f32, isOutput=True)

    with tile.TileContext(nc, num_cores=ncores) as tc:
        with (
            tc.tile_pool(name="dram", bufs=1, space="DRAM") as dpool,
            tc.tile_pool(name="const", bufs=1) as cpool,
            tc.tile_pool(name="state", bufs=1) as spool,
            tc.tile_pool(name="mslab", bufs=6) as mpool,
            tc.tile_pool(name="sslab", bufs=6) as sbpool,
            tc.tile_pool(name="ttile", bufs=4) as tpool,
            tc.tile_pool(name="psum", bufs=4, space="PSUM") as ppool,
            tc.tile_pool(name="psum2", bufs=2, space="PSUM") as ppool2,
        ):
            ag_ins = [dpool.tile([qsize[q], HIDDEN], f32, name=f"agin{q}") for q in range(NCHUNK)]
            tables = [dpool.tile([chunk_rows_q[q], HIDDEN], f32, name=f"table{q}") for q in range(NCHUNK)]
            bn_in = dpool.tile([HIDDEN, 2], f32)
            bn_out = dpool.tile([HIDDEN, 2], f32)

            # ---- constants to SBUF ----
            idxs = cpool.tile([P, S_total // 16], mybir.dt.int16)
            nc.sync.dma_start(out=idxs[:], in_=idx_in[:])
            dstl = cpool.tile([P, T_total], b16)
            nc.sync.dma_start(out=dstl[:], in_=dstl_in[:])
            nrm = cpool.tile([P, T_total], f32)
            nc.sync.dma_start(out=nrm[:], in_=nrm_in[:])
            d2 = cpool.tile([P, cells, P], b16)
            nc.sync.dma_start(out=d2[:], in_=d2_in.rearrange("p (q j) -> p q j", j=P))
            w0 = cpool.tile([IN_DIM, HIDDEN], f32)
            nc.sync.dma_start(out=w0[:], in_=w0_in[:])
            wls = []
            for i in range(3):
                wli = cpool.tile([HIDDEN, HIDDEN], b16, name=f"wl{i}")
                nc.sync.dma_start(out=wli[:], in_=wl_in[i])
                wls.append(wli)
            gb = cpool.tile([HIDDEN, 8], f32)
            nc.sync.dma_start(out=gb[:], in_=gb_in[:])
            hw1 = cpool.tile([HIDDEN, HIDDEN], b16)
            nc.sync.dma_start(out=hw1[:], in_=hw1_in[:])
            hb1 = cpool.tile([HIDDEN, 1], f32)
            nc.sync.dma_start(out=hb1[:], in_=hb1_in[:])
            hw2 = cpool.tile([HIDDEN, 1], b16)
            nc.sync.dma_start(out=hw2[:], in_=hw2_in[:])
            iota = cpool.tile([P, P], b16)
            nc.sync.dma_start(out=iota[:], in_=iota_in[:])

            # ---- state ----
            ZT = spool.tile([HIDDEN, NPC], f32)
            HT = spool.tile([HIDDEN, NPC], b16)
            shard_b = spool.tile([P, cells, HIDDEN], b16)
            stats = spool.tile([HIDDEN, 8], f32)  # sum, sumsq, mean, ex2, var, rstd, scale, shift
            NSQ = (NPC + 1023) // 1024
            sqp = spool.tile([HIDDEN, NSQ], f32)
            sq = spool.tile([HIDDEN, 1024], b16)

            for layer in range(4):
                # ---------- table phase: per-quarter tiles + early AllGather ----------
                for q in range(NCHUNK):
                    tq0 = sum(qtiles[:q])
                    for t in range(tq0, tq0 + qtiles[q]):
                        pt = ppool2.tile([P, HIDDEN], f32, tag="aux")
                        if layer == 0:
                            xt = tpool.tile([IN_DIM, P], f32, tag="xt")
                            nc.sync.dma_start(out=xt[:], in_=xT_in[:, t * P:(t + 1) * P])
                            nc.tensor.matmul(pt[:], lhsT=xt[:], rhs=w0[:], start=True, stop=True)
                        else:
                            nc.tensor.matmul(pt[:], lhsT=HT[:, t * P:(t + 1) * P],
                                             rhs=wls[layer - 1][:], start=True, stop=True)
                        tt = tpool.tile([P, HIDDEN], f32, tag="tt")
                        nc.vector.tensor_copy(out=tt[:], in_=pt[:])
                        nc.vector.tensor_copy(out=shard_b[:, t, :], in_=pt[:])
                        lo = t * P - qstart[q]
                        hi = min(P, qsize[q] - lo)
                        if hi > 0:
                            nc.sync.dma_start(out=ag_ins[q][lo:lo + hi, :], in_=tt[:hi, :])
                    nc.gpsimd.collective_compute(
                        "AllGather", mybir.AluOpType.bypass,
                        ins=[ag_ins[q].opt()], outs=[tables[q].opt()],
                        replica_groups=[list(range(ncores))],
                    )

                # ---------- self-loop diagonal pass (covers/initializes all ZT) ----------
                for q in range(cells):
                    zq = ppool.tile([HIDDEN, P], f32, tag="zc", name=f"zd_{layer}_{q}")
                    nc.tensor.matmul(zq[:], lhsT=shard_b[:, q, :], rhs=d2[:, q, :],
                                     start=True, stop=True)
                    nc.vector.tensor_copy(out=ZT[:, q * P:(q + 1) * P], in_=zq[:])

                # ---------- gather + scatter ----------
                for k in range(NCHUNK):
                    rb_slots = region_base[k]
                    ncalls = region_slots_pad[k] // SLAB
                    for j in range(ncalls):
                        s0 = rb_slots + j * SLAB          # first slot of call
                        t0 = s0 // P                       # first tile index
                        col0 = s0 // 16                    # idx col base
                        mt = mpool.tile([P, SLAB_T, HIDDEN], f32, tag="m")
                        nc.gpsimd.dma_gather(
                            out_ap=mt[:],
                            in_ap=tables[k][:],
                            idxs_ap=idxs[:, col0:col0 + SLAB // 16],
                            num_idxs=SLAB,
                            num_idxs_reg=SLAB,
                            elem_size=HIDDEN,
                            queue_num=(k * 1000 + j) % 2,
                        )
                        mb = mpool.tile([P, SLAB_T, HIDDEN], b16, tag="mb")
                        nc.vector.tensor_tensor(
                            out=mb[:], in0=mt[:],
                            in1=nrm[:, t0:t0 + SLAB_T].rearrange("p (t o) -> p t o", o=1).to_broadcast([P, SLAB_T, HIDDEN]),
                            op=mult_,
                        )
                        st = sbpool.tile([P, SLAB_T, P], b16, tag="s")
                        nc.vector.tensor_tensor(
                            out=st[:],
                            in0=dstl[:, t0:t0 + SLAB_T].rearrange("p (t o) -> p t o", o=1).to_broadcast([P, SLAB_T, P]),
                            in1=iota[:].rearrange("p (o f) -> p o f", o=1).to_broadcast([P, SLAB_T, P]),
                            op=iseq_,
                        )
                        for ti in range(SLAB_T):
                            tt_ = t0 + ti
                            w0 = tile_win[tt_] if tt_ < T_total else None
                            if w0 is None:
                                continue
                            is_first, is_last = tile_flags[tt_]
                            if is_first:
                                cur_zc = ppool.tile([HIDDEN, P], f32, tag="zc",
                                                    name=f"zc_{layer}_{k}_{j}_{ti}")
                            nc.tensor.matmul(cur_zc[:], lhsT=mb[:, ti, :], rhs=st[:, ti, :],
                                             start=is_first, stop=is_last)
                            if is_last:
                                nc.vector.tensor_tensor(out=ZT[:, w0:w0 + P],
                                                        in0=ZT[:, w0:w0 + P],
                                                        in1=cur_zc[:], op=add_)

                # ---------- BatchNorm + ReLU ----------
                nc.vector.reduce_sum(stats[:, 0:1], ZT[:], axis=mybir.AxisListType.X)
                for sc in range(NSQ):
                    w = min(1024, NPC - sc * 1024)
                    nc.vector.tensor_tensor(out=sq[:, :w], in0=ZT[:, sc * 1024:sc * 1024 + w],
                                            in1=ZT[:, sc * 1024:sc * 1024 + w], op=mult_)
                    nc.vector.reduce_sum(sqp[:, sc:sc + 1], sq[:, :w], axis=mybir.AxisListType.X)
                nc.vector.reduce_sum(stats[:, 1:2], sqp[:], axis=mybir.AxisListType.X)
                bt = tpool.tile([HIDDEN, 2], f32, tag="bt")
                nc.vector.tensor_copy(out=bt[:], in_=stats[:, 0:2])
                nc.sync.dma_start(out=bn_in[:], in_=bt[:])
                nc.gpsimd.collective_compute(
                    "AllReduce", add_,
                    ins=[bn_in.opt()], outs=[bn_out.opt()],
                    replica_groups=[list(range(ncores))],
                )
                nc.sync.dma_start(out=stats[:, 0:2], in_=bn_out[:])
                inv_n = 1.0 / float(n_nodes)
                nc.vector.tensor_scalar_mul(stats[:, 2:3], stats[:, 0:1], inv_n)   # mean
                nc.vector.tensor_scalar_mul(stats[:, 3:4], stats[:, 1:2], inv_n)   # E[x^2]
                nc.vector.tensor_tensor(out=stats[:, 4:5], in0=stats[:, 2:3],
                                        in1=stats[:, 2:3], op=mult_)               # mean^2
                nc.vector.tensor_tensor(out=stats[:, 4:5], in0=stats[:, 3:4],
                                        in1=stats[:, 4:5], op=subtract_)           # var
                nc.vector.tensor_scalar_add(stats[:, 4:5], stats[:, 4:5], EPS)
                nc.scalar.activation(stats[:, 5:6], stats[:, 4:5],
                                     mybir.ActivationFunctionType.Sqrt)             # std
                nc.vector.reciprocal(stats[:, 5:6], stats[:, 5:6])                  # rstd
                nc.vector.tensor_tensor(out=stats[:, 6:7], in0=gb[:, layer:layer + 1],
                                        in1=stats[:, 5:6], op=mult_)               # scale
                nc.vector.tensor_tensor(out=stats[:, 7:8], in0=stats[:, 2:3],
                                        in1=stats[:, 6:7], op=mult_)               # mean*scale
                nc.vector.tensor_tensor(out=stats[:, 7:8], in0=gb[:, 4 + layer:5 + layer],
                                        in1=stats[:, 7:8], op=subtract_)           # shift
                nc.vector.scalar_tensor_tensor(
                    out=HT[:], in0=ZT[:], scalar=stats[:, 6:7],
                    in1=stats[:, 7:8].to_broadcast([HIDDEN, NPC]),
                    op0=mult_, op1=add_,
                )
                nc.vector.tensor_scalar(HT[:], HT[:], 0.0, None, op0=max_)

            # ---------- head (512-column groups) ----------
            GW = 512
            ngrp = (NPC + GW - 1) // GW
            for g in range(ngrp):
                c0 = g * GW
                w = min(GW, NPC - c0)
                gp = ppool2.tile([HIDDEN, GW], f32, tag="aux")
                nc.tensor.matmul(gp[:, :w], lhsT=hw1[:], rhs=HT[:, c0:c0 + w],
                                 start=True, stop=True)
                gs = tpool.tile([HIDDEN, GW], b16, tag="gs")
                nc.vector.tensor_tensor(out=gs[:, :w], in0=gp[:, :w],
                                        in1=hb1[:].to_broadcast([HIDDEN, w]), op=add_)
                nc.vector.tensor_scalar(gs[:, :w], gs[:, :w], 0.0, None, op0=max_)
                op_ = ppool2.tile([1, GW], f32, tag="aux2")
                nc.tensor.matmul(op_[:, :w], lhsT=hw2[:], rhs=gs[:, :w],
                                 start=True, stop=True)
                ot = tpool.tile([1, GW], f32, tag="ot")
                nc.vector.tensor_scalar(ot[:, :w], op_[:, :w], hb2, None, op0=add_)
                hi = min(w, n_shard - c0)
                if hi > 0:
                    nc.sync.dma_start(out=y_out[c0:c0 + hi, :].rearrange("n o -> o n"),
                                      in_=ot[:, :hi])

    nc.finalize()
    _split_big_waits(nc)
    return nc


_CACHE = {}


def kernel(x, edge_index, W0, b0, Ws, bs, gammas, betas, hW1, hb1, hW2, hb2):
    sys.path.insert(0, os.path.dirname(os.path.abspath(__file__)))
    from concourse.bass_utils import run_bass_kernel_spmd

    in_maps, meta = host_prep(x, edge_index, W0, b0, Ws, bs, gammas, betas,
                              hW1, hb1, hW2, hb2)
    key = meta["S_total"]
    if key not in _CACHE:
        _CACHE[key] = build_program(meta)
    nc = _CACHE[key]

    trace = os.environ.get("BASS_GCN_TRACE", "0") == "1"
    kwargs = {}
    if trace:
        import tempfile
        _enable_axon_trace()
        kwargs = dict(trace=True, tmpdir=tempfile.mkdtemp())
    res = run_bass_kernel_spmd(nc, in_maps, core_ids=list(range(NCORES)), **kwargs)
    if trace:
        kernel.last_exec_time_ns = res.exec_time_ns
        kernel.last_trace_dir = kwargs["tmpdir"]
    out = np.concatenate([res.results[c]["y"] for c in range(NCORES)], axis=0)
    return out.astype(np.float32)


# revision 17
# speedup vs baseline: 1.5396x; 1.0408x over previous
"""GCN (4-layer GCNConv + BatchNorm + ReLU, MLP head) on 8 Trainium2 NeuronCores.

Strategy (see DESIGN.md):
- dst-shard nodes across 8 cores (12500 each); weights replicated.
- Per layer: table = H @ W for all nodes, assembled via AllGather of per-core
  shards; per-edge messages fetched with gpsimd.dma_gather (1024-idx calls,
  4x25000-row table chunks for int16 locals); scaled by edge norm on DVE
  (fp32->bf16); segment-sum by destination via one-hot scatter-matmuls into
  PSUM (S built on DVE by iota-compare); self-loops via per-cell diagonal
  matmuls. BatchNorm stats via free-axis reduction + 8-core AllReduce.
- Z/H kept feature-major ([64, nodes]) in SBUF so BN is per-partition.
"""

import os
import sys
import types

sys.path.insert(0, "/opt/trn_rl_repo")
if "/root/.axon_site" not in sys.path:
    sys.path.insert(0, "/root/.axon_site")

import numpy as np
import ml_dtypes

bf16 = ml_dtypes.bfloat16

def _split_big_waits(nc, limit=1):
    """walrus accepts only one sync-wait per instruction; move overflow waits
    onto preceding EventSemaphore ops on the same engine."""
    import concourse.mybir as mybir
    n_split = 0
    for blk in nc.main_func.blocks:
        i = 0
        while i < len(blk.instructions):
            inst = blk.instructions[i]
            si = inst.sync_info
            if si is not None and si.on_wait and len(si.on_wait) > limit:
                waits = list(si.on_wait)
                overflow, keep = waits[:-limit], waits[-limit:]
                idx = i
                for j in range(0, len(overflow), limit):
                    chunk = overflow[j:j + limit]
                    nop = mybir.InstEventSemaphore(
                        name=nc.get_next_instruction_name(), ins=[], outs=[])
                    nop.engine = inst.engine
                    nop.sync_info = mybir.SyncInfo(on_wait=chunk, on_update=[])
                    nc.register_instruction(nop)
                    blk.instructions.insert(idx, nop)
                    idx += 1
                    i += 1
                si.on_wait = keep
                n_split += 1
            i += 1
    return n_split


def _enable_axon_trace():
    """Register the antenv.axon_hooks NTFF shim (profiling under axon)."""
    import antenv
    if "antenv.axon_hooks" in sys.modules:
        return
    mod = types.ModuleType("antenv.axon_hooks")
    _hook = {"h": None}
    mod.set_axon_ntff_profile_hook = lambda h: _hook.__setitem__("h", h)
    mod.get_axon_ntff_profile_hook = lambda: _hook["h"]
    sys.modules["antenv.axon_hooks"] = mod
    antenv.axon_hooks = mod
    from trn_agent_boot.trn_boot import _ntff_profile_via_ctypes
    h = _ntff_profile_via_ctypes("/opt/axon/libaxon_pjrt.so")
    assert h is not None
    mod.set_axon_ntff_profile_hook(h)
    import concourse.bass_utils as bass_utils
    bass_utils.upload_artifacts = lambda tmpdir: str(tmpdir)


N_NODES = 100000
N_EDGES = 1600000
IN_DIM = 128
HIDDEN = 64
EPS = 1e-5
NCORES = 8
P = 128
NCHUNK = 4
SLAB = 1024          # idx per dma_gather call (HW ring limit ~1024)
SLAB_T = SLAB // P   # 8 tiles per slab


def _wrap_idx16(idx):
    """[n] -> [128, n//16] int16 wrapped in 16 partitions, replicated x8."""
    n16 = len(idx) // 16
    w = np.asarray(idx, np.int16).reshape(n16, 16).T
    return np.tile(w, (8, 1))


def host_prep(x, edge_index, W0, b0, Ws, bs, gammas, betas, hW1, hb1, hW2, hb2,
              n_nodes=N_NODES, ncores=NCORES):
    """Shard + preprocess. Returns (in_maps, meta) for the SPMD program."""
    n_shard = n_nodes // ncores
    cells = (n_shard + P - 1) // P            # dst cells per core
    # shard quarters (tile-aligned except the last): chunk q of the table is the
    # AllGather of every core's quarter q, laid out rank-major.
    qtiles = [25, 25, 25, cells - 75] if cells == 98 else [cells // 4] * 3 + [cells - 3 * (cells // 4)]
    qsize = []
    qstart = [0]
    for qi in range(NCHUNK):
        hi = min(n_shard, (qstart[-1] + qtiles[qi] * P))
        qsize.append(hi - qstart[-1])
        qstart.append(hi)
    qstart = qstart[:NCHUNK]
    chunk_rows_q = [ncores * s for s in qsize]

    src = np.asarray(edge_index[0], np.int64)
    dst = np.asarray(edge_index[1], np.int64)
    deg = np.bincount(dst, minlength=n_nodes).astype(np.float64) + 1.0  # + self loop
    dinv = (1.0 / np.sqrt(deg)).astype(np.float64)
    dinv2 = (dinv * dinv).astype(np.float32)

    core_of = dst // n_shard
    src_r = src // n_shard
    src_j = src % n_shard
    chunk_of = np.searchsorted(np.array(qstart[1:] + [n_shard]), src_j, side="right")
    # local row of src within its chunk's rank-major table
    qs = np.array(qstart)[chunk_of]
    qz = np.array(qsize)[chunk_of]
    src_local = src_r * qz + (src_j - qs)

    BLK = 64                                   # dsts per budget block
    nblk = (n_shard + BLK - 1) // BLK          # blocks per core (per chunk)
    dstl_all = dst - core_of * n_shard
    blk_of = dstl_all // BLK
    counts = np.zeros((ncores, NCHUNK, nblk), np.int64)
    np.add.at(counts, (core_of, chunk_of, blk_of), 1)
    budget = np.maximum(counts.max(axis=0), P)  # [NCHUNK, nblk] slots, >= 128

    # slot layout: chunk-major regions; block b gets budget[k, b] slots
    region_slots = budget.sum(axis=1)
    region_slots_pad = -(-region_slots // SLAB) * SLAB
    S_total = int(region_slots_pad.sum())
    T_total = S_total // P

    blk_base = np.zeros((NCHUNK, nblk), np.int64)
    base = 0
    region_base = []
    for k in range(NCHUNK):
        region_base.append(base)
        for b in range(nblk):
            blk_base[k, b] = base
            base += int(budget[k, b])
        base = region_base[k] + int(region_slots_pad[k])
    assert base == S_total

    # per-tile dst window (uniform across cores): win0 = BLK * block(first slot),
    # clamped; None for region-tail tiles holding no block slots
    cells = (n_shard + P - 1) // P
    NPC = cells * P
    slot_block = np.full(S_total, -1, np.int64)   # block of each slot (k-local)
    for k in range(NCHUNK):
        for b in range(nblk):
            s0 = int(blk_base[k, b])
            slot_block[s0:s0 + int(budget[k, b])] = b
    tile_win = []
    region_of_tile = []
    for t in range(T_total):
        b = slot_block[t * P]
        tile_win.append(None if b < 0 else int(min(b * BLK, NPC - P)))
        # which chunk region this tile is in
        k_of = 0
        for k in range(NCHUNK):
            if t * P >= region_base[k]:
                k_of = k
        region_of_tile.append(k_of)
    # chains: consecutive tiles, same window, same region
    tile_flags = []  # (is_first, is_last) per tile (None for tail tiles)
    for t in range(T_total):
        if tile_win[t] is None:
            tile_flags.append(None)
            continue
        prev_same = (t > 0 and tile_win[t - 1] == tile_win[t]
                     and region_of_tile[t - 1] == region_of_tile[t])
        next_same = (t + 1 < T_total and tile_win[t + 1] == tile_win[t]
                     and region_of_tile[t + 1] == region_of_tile[t])
        tile_flags.append((not prev_same, not next_same))

    in_maps = []
    for c in range(ncores):
        m = core_of == c
        src_c = src[m]
        srcl_c = src_local[m]
        dstl_c = dstl_all[m]
        k_c = chunk_of[m]
        b_c = blk_of[m]
        norm_c = (dinv[src_c] * dinv[dst[m]]).astype(np.float32)

        order = np.lexsort((dstl_c, b_c, k_c))
        src_c, srcl_c, dstl_c, k_c, b_c, norm_c = (
            a[order] for a in (src_c, srcl_c, dstl_c, k_c, b_c, norm_c))

        idx_slots = np.zeros(S_total, np.int16)
        dl_slots = np.full(S_total, 255.0, np.float32)
        nm_slots = np.zeros(S_total, np.float32)
        ptr = 0
        for k in range(NCHUNK):
            cnts = counts[c, k]
            for b in range(nblk):
                n_kb = int(cnts[b])
                sl = slice(ptr, ptr + n_kb)
                s0 = int(blk_base[k, b])
                idx_slots[s0:s0 + n_kb] = srcl_c[sl].astype(np.int16)
                # dst_local relative to the WINDOW of each slot's tile
                ss = np.arange(s0, s0 + n_kb)
                w0 = np.array([tile_win[t] for t in ss // P], np.int64)
                dl_slots[s0:s0 + n_kb] = (dstl_c[sl] - w0).astype(np.float32)
                nm_slots[s0:s0 + n_kb] = norm_c[sl]
                ptr += n_kb
        assert ptr == len(src_c)
        assert (dl_slots >= 0).all() and (dl_slots <= 255).all()

        # slot s -> [s % 128, s // 128]
        idxw = np.concatenate(
            [_wrap_idx16(idx_slots[region_base[k]:region_base[k] + int(region_slots_pad[k])])
             for k in range(NCHUNK)], axis=1)
        # NOTE: wrapped layout is per-16 within each slab of its region; dma_gather
        # unwraps [16, :] as (s p): slot i of a call -> idx at [i % 16, i // 16] of the
        # call's column slice. Building per-region keeps call slices aligned.
        dl_arr = dl_slots.reshape(T_total, P).T.astype(bf16)      # [128, T]
        nm_arr = nm_slots.reshape(T_total, P).T.astype(np.float32)  # [128, T] fp32

        # D2: [128, cells*128] bf16; [r, q*128 + j] = (r == j) * dinv2[shard node q*128+r]
        d2 = np.zeros((P, cells * P), np.float32)
        for q in range(cells):
            nmax = min(P, n_shard - q * P)
            rr = np.arange(nmax)
            d2[rr, q * P + rr] = dinv2[c * n_shard + q * P + rr]
        # x shard transposed, padded to cells*128 columns
        xT = np.zeros((IN_DIM, cells * P), np.float32)
        xT[:, :n_shard] = np.asarray(x, np.float32)[c * n_shard:(c + 1) * n_shard].T

        in_maps.append({
            "xT": xT,
            "idxw": np.ascontiguousarray(idxw),
            "dstl": np.ascontiguousarray(dl_arr),
            "nrm": np.ascontiguousarray(nm_arr),
            "d2": d2.astype(bf16),
            "w0": np.asarray(W0, np.float32),
            "wl": np.asarray(Ws, np.float32).astype(bf16),
            "gb": np.concatenate([np.asarray(gammas, np.float32).T,
                                  np.asarray(betas, np.float32).T], axis=1),  # [64, 8]
            "hw1": np.asarray(hW1, np.float32).astype(bf16),
            "hb1": np.asarray(hb1, np.float32).reshape(HIDDEN, 1),
            "hw2": np.asarray(hW2, np.float32).astype(bf16).reshape(HIDDEN, 1),
            "iota": np.tile(np.arange(P, dtype=np.float32).reshape(1, P), (P, 1)).astype(bf16),
        })

    meta = dict(n_nodes=n_nodes, n_shard=n_shard, cells=cells,
                S_total=S_total, T_total=T_total, tile_win=tile_win,
                tile_flags=tile_flags, qtiles=qtiles, qsize=qsize, qstart=qstart,
                chunk_rows_q=chunk_rows_q,
                region_base=region_base,
                region_slots_pad=[int(v) for v in region_slots_pad],
                hb2=float(np.asarray(hb2).reshape(-1)[0]))
    return in_maps, meta


def build_program(meta, ncores=NCORES):
    import concourse.bass as bass
    import concourse.bacc as bacc
    import concourse.mybir as mybir
    import concourse.tile as tile

    n_nodes = meta["n_nodes"]
    n_shard = meta["n_shard"]
    cells = meta["cells"]
    qtiles = meta["qtiles"]
    qsize = meta["qsize"]
    qstart = meta["qstart"]
    chunk_rows_q = meta["chunk_rows_q"]
    S_total = meta["S_total"]
    T_total = meta["T_total"]
    tile_win = meta["tile_win"]
    tile_flags = meta["tile_flags"]
    region_base = meta["region_base"]
    region_slots_pad = meta["region_slots_pad"]
    hb2 = meta["hb2"]
    NPC = cells * P  # padded shard width

    f32 = mybir.dt.float32
    b16 = mybir.dt.bfloat16
    add_ = mybir.AluOpType.add
    mult_ = mybir.AluOpType.mult
    iseq_ = mybir.AluOpType.is_equal
    max_ = mybir.AluOpType.max
    subtract_ = mybir.AluOpType.subtract

    nc = bacc.Bacc(num_devices=ncores, num_swdge_queues=2)
    xT_in = nc.declare_dram_parameter("xT", [IN_DIM, NPC], f32, isOutput=False)
    idx_in = nc.declare_dram_parameter("idxw", [P, S_total // 16], mybir.dt.int16, isOutput=False)
    dstl_in = nc.declare_dram_parameter("dstl", [P, T_total], b16, isOutput=False)
    nrm_in = nc.declare_dram_parameter("nrm", [P, T_total], f32, isOutput=False)
    d2_in = nc.declare_dram_parameter("d2", [P, cells * P], b16, isOutput=False)
    w0_in = nc.declare_dram_parameter("w0", [IN_DIM, HIDDEN], f32, isOutput=False)
    wl_in = nc.declare_dram_parameter("wl", [3, HIDDEN, HIDDEN], b16, isOutput=False)
    gb_in = nc.declare_dram_parameter("gb", [HIDDEN, 8], f32, isOutput=False)
    hw1_in = nc.declare_dram_parameter("hw1", [HIDDEN, HIDDEN], b16, isOutput=False)
    hb1_in = nc.declare_dram_parameter("hb1", [HIDDEN, 1], f32, isOutput=False)
    hw2_in = nc.declare_dram_parameter("hw2", [HIDDEN, 1], b16, isOutput=False)
    iota_in = nc.declare_dram_parameter("iota", [P, P], b16, isOutput=False)
    y_out = nc.declare_dram_parameter("y", [n_shard, 1], f32, isOutput=True)

    with tile.TileContext(nc, num_cores=ncores) as tc:
        with (
            tc.tile_pool(name="dram", bufs=1, space="DRAM") as dpool,
            tc.tile_pool(name="const", bufs=1) as cpool,
            tc.tile_pool(name="state", bufs=1) as spool,
            tc.tile_pool(name="mslab", bufs=6) as mpool,
            tc.tile_pool(name="sslab", bufs=6) as sbpool,
            tc.tile_pool(name="ttile", bufs=4) as tpool,
            tc.tile_pool(name="psum", bufs=4, space="PSUM") as ppool,
            tc.tile_pool(name="psum2", bufs=2, space="PSUM") as ppool2,
        ):
            ag_ins = [dpool.tile([qsize[q], HIDDEN], f32, name=f"agin{q}") for q in range(NCHUNK)]
            tables = [dpool.tile([chunk_rows_q[q], HIDDEN], f32, name=f"table{q}") for q in range(NCHUNK)]
            bn_in = dpool.tile([HIDDEN, 2], f32)
            bn_out = dpool.tile([HIDDEN, 2], f32)

            # ---- constants to SBUF ----
            idxs = cpool.tile([P, S_total // 16], mybir.dt.int16)
            nc.sync.dma_start(out=idxs[:], in_=idx_in[:])
            dstl = cpool.tile([P, T_total], b16)
            nc.sync.dma_start(out=dstl[:], in_=dstl_in[:])
            nrm = cpool.tile([P, T_total], f32)
            nc.sync.dma_start(out=nrm[:], in_=nrm_in[:])
            d2 = cpool.tile([P, cells, P], b16)
            nc.sync.dma_start(out=d2[:], in_=d2_in.rearrange("p (q j) -> p q j", j=P))
            w0 = cpool.tile([IN_DIM, HIDDEN], f32)
            nc.sync.dma_start(out=w0[:], in_=w0_in[:])
            wls = []
            for i in range(3):
                wli = cpool.tile([HIDDEN, HIDDEN], b16, name=f"wl{i}")
                nc.sync.dma_start(out=wli[:], in_=wl_in[i])
                wls.append(wli)
            gb = cpool.tile([HIDDEN, 8], f32)
            nc.sync.dma_start(out=gb[:], in_=gb_in[:])
            hw1 = cpool.tile([HIDDEN, HIDDEN], b16)
            nc.sync.dma_start(out=hw1[:], in_=hw1_in[:])
            hb1 = cpool.tile([HIDDEN, 1], f32)
            nc.sync.dma_start(out=hb1[:], in_=hb1_in[:])
            hw2 = cpool.tile([HIDDEN, 1], b16)
            nc.sync.dma_start(out=hw2[:], in_=hw2_in[:])
            iota = cpool.tile([P, P], b16)
            nc.sync.dma_start(out=iota[:], in_=iota_in[:])

            # ---- state ----
            ZT = spool.tile([HIDDEN, NPC], f32)
            HT = spool.tile([HIDDEN, NPC], b16)
            shard_b = spool.tile([P, cells, HIDDEN], b16)
            stats = spool.tile([HIDDEN, 8], f32)  # sum, sumsq, mean, ex2, var, rstd, scale, shift
            NSQ = (NPC + 1023) // 1024
            sqp = spool.tile([HIDDEN, NSQ], f32)
            sq = spool.tile([HIDDEN, 1024], b16)

            for layer in range(4):
                # ---------- table phase: per-quarter tiles + early AllGather ----------
                for q in range(NCHUNK):
                    tq0 = sum(qtiles[:q])
                    for t in range(tq0, tq0 + qtiles[q]):
                        pt = ppool2.tile([P, HIDDEN], f32, tag="aux")
                        if layer == 0:
                            xt = tpool.tile([IN_DIM, P], f32, tag="xt")
                            nc.sync.dma_start(out=xt[:], in_=xT_in[:, t * P:(t + 1) * P])
                            nc.tensor.matmul(pt[:], lhsT=xt[:], rhs=w0[:], start=True, stop=True)
                        else:
                            nc.tensor.matmul(pt[:], lhsT=HT[:, t * P:(t + 1) * P],
                                             rhs=wls[layer - 1][:], start=True, stop=True)
                        tt = tpool.tile([P, HIDDEN], f32, tag="tt")
                        nc.vector.tensor_copy(out=tt[:], in_=pt[:])
                        nc.vector.tensor_copy(out=shard_b[:, t, :], in_=pt[:])
                        zq = ppool.tile([HIDDEN, P], f32, tag="zc", name=f"zd_{layer}_{t}")
                        nc.tensor.matmul(zq[:], lhsT=shard_b[:, t, :], rhs=d2[:, t, :],
                                         start=True, stop=True)
                        nc.vector.tensor_copy(out=ZT[:, t * P:(t + 1) * P], in_=zq[:])
                        lo = t * P - qstart[q]
                        hi = min(P, qsize[q] - lo)
                        if hi > 0:
                            nc.sync.dma_start(out=ag_ins[q][lo:lo + hi, :], in_=tt[:hi, :])
                    nc.gpsimd.collective_compute(
                        "AllGather", mybir.AluOpType.bypass,
                        ins=[ag_ins[q].opt()], outs=[tables[q].opt()],
                        replica_groups=[list(range(ncores))],
                    )

                # ---------- gather + scatter ----------
                for k in range(NCHUNK):
                    rb_slots = region_base[k]
                    ncalls = region_slots_pad[k] // SLAB
                    for j in range(ncalls):
                        s0 = rb_slots + j * SLAB          # first slot of call
                        t0 = s0 // P                       # first tile index
                        col0 = s0 // 16                    # idx col base
                        mt = mpool.tile([P, SLAB_T, HIDDEN], f32, tag="m")
                        nc.gpsimd.dma_gather(
                            out_ap=mt[:],
                            in_ap=tables[k][:],
                            idxs_ap=idxs[:, col0:col0 + SLAB // 16],
                            num_idxs=SLAB,
                            num_idxs_reg=SLAB,
                            elem_size=HIDDEN,
                            queue_num=(k * 1000 + j) % 2,
                        )
                        mb = mpool.tile([P, SLAB_T, HIDDEN], b16, tag="mb")
                        nc.vector.tensor_tensor(
                            out=mb[:], in0=mt[:],
                            in1=nrm[:, t0:t0 + SLAB_T].rearrange("p (t o) -> p t o", o=1).to_broadcast([P, SLAB_T, HIDDEN]),
                            op=mult_,
                        )
                        st = sbpool.tile([P, SLAB_T, P], b16, tag="s")
                        nc.vector.tensor_tensor(
                            out=st[:],
                            in0=dstl[:, t0:t0 + SLAB_T].rearrange("p (t o) -> p t o", o=1).to_broadcast([P, SLAB_T, P]),
                            in1=iota[:].rearrange("p (o f) -> p o f", o=1).to_broadcast([P, SLAB_T, P]),
                            op=iseq_,
                        )
                        for ti in range(SLAB_T):
                            tt_ = t0 + ti
                            w0 = tile_win[tt_] if tt_ < T_total else None
                            if w0 is None:
                                continue
                            is_first, is_last = tile_flags[tt_]
                            if is_first:
                                cur_zc = ppool.tile([HIDDEN, P], f32, tag="zc",
                                                    name=f"zc_{layer}_{k}_{j}_{ti}")
                            nc.tensor.matmul(cur_zc[:], lhsT=mb[:, ti, :], rhs=st[:, ti, :],
                                             start=is_first, stop=is_last)
                            if is_last:
                                nc.vector.tensor_tensor(out=ZT[:, w0:w0 + P],
                                                        in0=ZT[:, w0:w0 + P],
                                                        in1=cur_zc[:], op=add_)

                # ---------- BatchNorm + ReLU ----------
                nc.vector.reduce_sum(stats[:, 0:1], ZT[:], axis=mybir.AxisListType.X)
                for sc in range(NSQ):
                    w = min(1024, NPC - sc * 1024)
                    nc.vector.tensor_tensor(out=sq[:, :w], in0=ZT[:, sc * 1024:sc * 1024 + w],
                                            in1=ZT[:, sc * 1024:sc * 1024 + w], op=mult_)
                    nc.vector.reduce_sum(sqp[:, sc:sc + 1], sq[:, :w], axis=mybir.AxisListType.X)
                nc.vector.reduce_sum(stats[:, 1:2], sqp[:], axis=mybir.AxisListType.X)
                bt = tpool.tile([HIDDEN, 2], f32, tag="bt")
                nc.vector.tensor_copy(out=bt[:], in_=stats[:, 0:2])
                nc.sync.dma_start(out=bn_in[:], in_=bt[:])
                nc.gpsimd.collective_compute(
                    "AllReduce", add_,
                    ins=[bn_in.opt()], outs=[bn_out.opt()],
                    replica_groups=[list(range(ncores))],
                )
                nc.sync.dma_start(out=stats[:, 0:2], in_=bn_out[:])
                inv_n = 1.0 / float(n_nodes)
                nc.vector.tensor_scalar_mul(stats[:, 2:3], stats[:, 0:1], inv_n)   # mean
                nc.vector.tensor_scalar_mul(stats[:, 3:4], stats[:, 1:2], inv_n)   # E[x^2]
                nc.vector.tensor_tensor(out=stats[:, 4:5], in0=stats[:, 2:3],
                                        in1=stats[:, 2:3], op=mult_)               # mean^2
                nc.vector.tensor_tensor(out=stats[:, 4:5], in0=stats[:, 3:4],
                                        in1=stats[:, 4:5], op=subtract_)           # var
                nc.vector.tensor_scalar_add(stats[:, 4:5], stats[:, 4:5], EPS)
                nc.scalar.activation(stats[:, 5:6], stats[:, 4:5],
                                     mybir.ActivationFunctionType.Sqrt)             # std
                nc.vector.reciprocal(stats[:, 5:6], stats[:, 5:6])                  # rstd
                nc.vector.tensor_tensor(out=stats[:, 6:7], in0=gb[:, layer:layer + 1],
                                        in1=stats[:, 5:6], op=mult_)               # scale
                nc.vector.tensor_tensor(out=stats[:, 7:8], in0=stats[:, 2:3],
                                        in1=stats[:, 6:7], op=mult_)               # mean*scale
                nc.vector.tensor_tensor(out=stats[:, 7:8], in0=gb[:, 4 + layer:5 + layer],
                                        in1=stats[:, 7:8], op=subtract_)           # shift
                nc.vector.scalar_tensor_tensor(
                    out=HT[:], in0=ZT[:], scalar=stats[:, 6:7],
                    in1=stats[:, 7:8].to_broadcast([HIDDEN, NPC]),
                    op0=mult_, op1=add_,
                )
                nc.vector.tensor_scalar(HT[:], HT[:], 0.0, None, op0=max_)

            # ---------- head (512-column groups) ----------
            GW = 512
            ngrp = (NPC + GW - 1) // GW
            for g in range(ngrp):
                c0 = g * GW
                w = min(GW, NPC - c0)
                gp = ppool2.tile([HIDDEN, GW], f32, tag="aux")
                nc.tensor.matmul(gp[:, :w], lhsT=hw1[:], rhs=HT[:, c0:c0 + w],
                                 start=True, stop=True)
                gs = tpool.tile([HIDDEN, GW], b16, tag="gs")
                nc.vector.tensor_tensor(out=gs[:, :w], in0=gp[:, :w],
                                        in1=hb1[:].to_broadcast([HIDDEN, w]), op=add_)
                nc.vector.tensor_scalar(gs[:, :w], gs[:, :w], 0.0, None, op0=max_)
                op_ = ppool2.tile([1, GW], f32, tag="aux2")
                nc.tensor.matmul(op_[:, :w], lhsT=hw2[:], rhs=gs[:, :w],
                                 start=True, stop=True)
                ot = tpool.tile([1, GW], f32, tag="ot")
                nc.vector.tensor_scalar(ot[:, :w], op_[:, :w], hb2, None, op0=add_)
                hi = min(w, n_shard - c0)
                if hi > 0:
                    nc.sync.dma_start(out=y_out[c0:c0 + hi, :].rearrange("n o -> o n"),
                                      in_=ot[:, :hi])

    nc.finalize()
    _split_big_waits(nc)
    return nc


_CACHE = {}


def kernel(x, edge_index, W0, b0, Ws, bs, gammas, betas, hW1, hb1, hW2, hb2):
    sys.path.insert(0, os.path.dirname(os.path.abspath(__file__)))
    from concourse.bass_utils import run_bass_kernel_spmd

    in_maps, meta = host_prep(x, edge_index, W0, b0, Ws, bs, gammas, betas,
                              hW1, hb1, hW2, hb2)
    key = meta["S_total"]
    if key not in _CACHE:
        _CACHE[key] = build_program(meta)
    nc = _CACHE[key]

    trace = os.environ.get("BASS_GCN_TRACE", "0") == "1"
    kwargs = {}
    if trace:
        import tempfile
        _enable_axon_trace()
        kwargs = dict(trace=True, tmpdir=tempfile.mkdtemp())
    res = run_bass_kernel_spmd(nc, in_maps, core_ids=list(range(NCORES)), **kwargs)
    if trace:
        kernel.last_exec_time_ns = res.exec_time_ns
        kernel.last_trace_dir = kwargs["tmpdir"]
    out = np.concatenate([res.results[c]["y"] for c in range(NCORES)], axis=0)
    return out.astype(np.float32)
